# revision 11
# baseline (speedup 1.0000x reference)
"""Trainium2 Bass kernel for nn_DecoderRNN (attention LSTM decoder + vocab projection).

Strategy (8 NeuronCores):
  - The 63-step LSTM/attention recurrence is inherently sequential and its per-step
    matmul work does not shrink with batch sharding (B=128 <= one PE M-tile), while
    per-step collectives cost >= ~5us each — so the recurrence is REPLICATED on all
    cores (identical SPMD program).
  - The dominant output projection (T*B, H) x (H, V) is sharded over the vocab
    dimension: each core computes/writes its own V/8 = 1250 logit columns in-loop.
  - The per-timestep x-dependent GEMM inputs (attention x-part PA and the folded
    gates x-part PX = X @ (attd_Wx.T @ W_ih.T)) are precomputed SHARDED over t
    (8 steps/core) and exchanged with two one-time AllGathers.
  - attd/W_ih are folded: G = attended @ Ca + h @ W_hh.T + PX[t], with
    Ca = attd_Wa.T @ W_ih.T computed once on device.
  - Softmax normalization is deferred: attended_norm = exp(score) * cnn * (1/sum),
    with the sum taken via a ones-matmul over the feature-major exp tile.
  - Ragged lengths (sorted desc) are baked into the instruction stream: at step t
    only the first n_t rows update h/c and only those logit rows are written; the
    rest of the output is filled by DMAs from a zero tile.
"""

import os
import sys

import numpy as np

for _p in ("/opt/trn_rl_repo", "/root/.axon_site/_ro/trn_rl_repo"):
    if os.path.isdir(_p) and _p not in sys.path:
        sys.path.insert(0, _p)

import concourse.bass as bass
import concourse.tile as tile
from concourse import bacc
from concourse import mybir
from concourse.bass_utils import run_bass_kernel_spmd
from concourse.masks import make_identity

F32 = mybir.dt.float32
I32 = mybir.dt.int32
ADD = mybir.AluOpType.add
MULT = mybir.AluOpType.mult

B, T, E, H, A, V = 128, 64, 512, 512, 512, 10000
G4 = 4 * H                      # 2048
NCORES = 8
VS = V // NCORES                # 1250 vocab columns per core
TPC = 8                         # precompute t-steps per core (63 real + 1 pad)
P = 128

KE = E // P                     # 4 k-tiles over E
KH = H // P
KA = A // P
MA = A // P                     # A m-tiles (feature-major attention)
NCH = G4 // 512                 # 4 n-chunks of 512 over the gate dim


def _bcast_rows(dram_ap, n):
    """DMA source AP replicating a [1, N] DRAM row across n partitions."""
    return bass.AP(tensor=dram_ap.tensor, offset=dram_ap.offset,
                   ap=[[0, n]] + [list(x) for x in dram_ap.ap[1:]])


def _build_nc(n_t):
    """Build the SPMD Bass program. n_t[t] = number of active batch rows at step t
    (lengths sorted descending -> active rows are a prefix)."""
    nc = bacc.Bacc("TRN2", target_bir_lowering=False, debug=False,
                   num_devices=NCORES)

    # ---------------- I/O ----------------
    feat_T = nc.declare_dram_parameter("feat_T", [E, B], F32, isOutput=False)
    cnn_T = nc.declare_dram_parameter("cnn_T", [A, B], F32, isOutput=False)
    caps = nc.declare_dram_parameter("caps", [TPC, B], I32, isOutput=False)
    emb_W = nc.declare_dram_parameter("emb_W", [V, E], F32, isOutput=False)
    W_ih_T = nc.declare_dram_parameter("W_ih_T", [E, G4], F32, isOutput=False)
    W_hh_T = nc.declare_dram_parameter("W_hh_T", [H, G4], F32, isOutput=False)
    b0_row = nc.declare_dram_parameter("b0_row", [1, G4], F32, isOutput=False)
    attWh_T = nc.declare_dram_parameter("attWh_T", [H, A], F32, isOutput=False)
    attWx_T = nc.declare_dram_parameter("attWx_T", [E, A], F32, isOutput=False)
    att_b4 = nc.declare_dram_parameter("att_b4", [MA, P], F32, isOutput=False)
    attd_Wx = nc.declare_dram_parameter("attd_Wx", [E, E], F32, isOutput=False)
    attd_Wa = nc.declare_dram_parameter("attd_Wa", [E, A], F32, isOutput=False)
    attd_b4 = nc.declare_dram_parameter("attd_b4", [KE, P], F32, isOutput=False)
    out_WsT = nc.declare_dram_parameter("out_WsT", [H, VS], F32, isOutput=False)
    out_bs = nc.declare_dram_parameter("out_bs", [1, VS], F32, isOutput=False)
    out = nc.declare_dram_parameter("out", [T, B, VS], F32, isOutput=True)

    rows_sh = TPC * B           # 1024 precompute rows per core

    with tile.TileContext(nc) as tc:
        with (
            tc.tile_pool(name="dram", bufs=1, space="DRAM") as dramp,
            tc.tile_pool(name="consts", bufs=1) as consts,
            tc.tile_pool(name="state", bufs=1) as state,
            tc.tile_pool(name="ps_g", bufs=1, space="PSUM") as ps_g,
            tc.tile_pool(name="ps_s", bufs=1, space="PSUM") as ps_s,
            tc.tile_pool(name="ps_d", bufs=1, space="PSUM") as ps_d,
            tc.tile_pool(name="ps_t", bufs=1, space="PSUM") as ps_t,
            tc.tile_pool(name="ps_o", bufs=1, space="PSUM") as ps_o,
        ):
            # DRAM scratch
            px_sh = dramp.tile([rows_sh, G4], F32)
            pa_sh = dramp.tile([A, rows_sh], F32)
            px_all = dramp.tile([NCORES * rows_sh, G4], F32, addr_space="Shared")
            pa_all = dramp.tile([NCORES, A, rows_sh], F32, addr_space="Shared")
            ca_dram = dramp.tile([A, G4], F32)

            def load_tiled(dst, dram_ap, ktiles, ncols, nch=512):
                """dst [P, ktiles, ncols] <- dram [(ktiles*P), ncols], one DMA per
                (k, 512-col chunk) so consumers wait on few DMA semaphores."""
                for k in range(ktiles):
                    for n0 in range(0, ncols, nch):
                        n1 = min(n0 + nch, ncols)
                        nc.sync.dma_start(dst[:, k, n0:n1],
                                          dram_ap[k * P:(k + 1) * P, n0:n1])

            # ---------------- shared constants ----------------
            identity = consts.tile([P, P], F32)
            make_identity(nc, identity)
            zero_out = consts.tile([P, VS], F32)
            nc.vector.memset(zero_out, 0.0)
            ones_col = consts.tile([P, 1], F32)
            nc.vector.memset(ones_col, 1.0)
            cnn_sb = consts.tile([P, KA, B], F32)     # cnn_T feature-major
            load_tiled(cnn_sb, cnn_T[:, :], KA, B)
            attb_sb = consts.tile([P, MA], F32)
            nc.sync.dma_start(attb_sb, att_b4[:, :].rearrange("m p -> p m"))
            outb_bc = consts.tile([P, VS], F32)
            nc.sync.dma_start(outb_bc, _bcast_rows(out_bs[:, :], P))

            # recurrent state (lives across both phases)
            hT = state.tile([P, KH, B], F32)          # h transposed (feature-major)
            c_sb = state.tile([P, H], F32)            # c, B-major


            def load_tiled(dst, dram_ap, ktiles, ncols, nch=512):
                """dst [P, ktiles, ncols] <- dram [(ktiles*P), ncols], one DMA per
                (k, 512-col chunk) so consumers wait on few DMA semaphores."""
                for k in range(ktiles):
                    for n0 in range(0, ncols, nch):
                        n1 = min(n0 + nch, ncols)
                        nc.sync.dma_start(dst[:, k, n0:n1],
                                          dram_ap[k * P:(k + 1) * P, n0:n1])

            def g4_matmul(psg, lhs_list, rhs_list):
                """psg [P, G4] += sum_k lhs[k].T @ rhs[k] with N chunked to 512."""
                nk = len(lhs_list)
                for k in range(nk):
                    for n in range(NCH):
                        ns = slice(n * 512, (n + 1) * 512)
                        nc.tensor.matmul(psg[:, ns], lhs_list[k], rhs_list[k][:, ns],
                                         start=(k == 0), stop=(k == nk - 1))

            def lstm_pointwise(gsb, nt, first, pool):
                """gsb [P, 4H] pre-activation gates (B-major), activations computed
                in-place. Updates c_sb rows and hT cols [0:nt]."""
                r = slice(0, nt)
                SIG = mybir.ActivationFunctionType.Sigmoid
                TANH = mybir.ActivationFunctionType.Tanh
                nc.scalar.activation(gsb[r, 0:H], gsb[r, 0:H], SIG)              # i
                if not first:
                    nc.scalar.activation(gsb[r, H:2 * H], gsb[r, H:2 * H], SIG)  # f
                nc.scalar.activation(gsb[r, 2 * H:3 * H], gsb[r, 2 * H:3 * H], TANH)
                nc.scalar.activation(gsb[r, 3 * H:4 * H], gsb[r, 3 * H:4 * H], SIG)
                ig = pool.tile([P, H], F32, tag="ig")
                nc.vector.tensor_mul(ig[r, :], gsb[r, 0:H], gsb[r, 2 * H:3 * H])
                if first:
                    nc.vector.tensor_copy(c_sb[r, :], ig[r, :])
                else:
                    fc = pool.tile([P, H], F32, tag="fc")
                    nc.vector.tensor_mul(fc[r, :], gsb[r, H:2 * H], c_sb[r, :])
                    nc.vector.tensor_add(c_sb[r, :], fc[r, :], ig[r, :])
                tnc = pool.tile([P, H], F32, tag="tanhc")
                nc.scalar.activation(tnc[r, :], c_sb[r, :], TANH)
                h2 = pool.tile([P, H], F32, tag="h2")
                nc.vector.tensor_mul(h2[r, :], gsb[r, 3 * H:4 * H], tnc[r, :])
                for m in range(KH):
                    pst = ps_t.tile([P, P], F32, tag="pst")
                    nc.tensor.transpose(pst, h2[:, m * P:(m + 1) * P], identity)
                    nc.vector.tensor_copy(hT[:, m, 0:nt], pst[:, 0:nt])

            # ============ PHASE A: folds + PA/PX precompute + exchange + step 0 ============
            with tc.tile_pool(name="wpre", bufs=1) as wpre, \
                 tc.tile_pool(name="pre", bufs=2) as pre, \
                 tc.tile_pool(name="xtp", bufs=1) as xtp:
                awx_sb = wpre.tile([P, KE, A], F32)       # att_Wx.T (lhsT for PA)
                load_tiled(awx_sb, attWx_T[:, :], KE, A)
                wih_sb = wpre.tile([P, KE, G4], F32)      # W_ih.T (rhs)
                load_tiled(wih_sb, W_ih_T[:, :], KE, G4)
                adwx_sb = wpre.tile([P, KE, E], F32)      # attd_Wx (lhsT for Cx)
                load_tiled(adwx_sb, attd_Wx[:, :], KE, E)
                adwa_sb = wpre.tile([P, KE, A], F32)      # attd_Wa (lhsT for Ca)
                load_tiled(adwa_sb, attd_Wa[:, :], KE, A)
                attdb_sb = wpre.tile([P, KE], F32)
                nc.sync.dma_start(attdb_sb, attd_b4[:, :].rearrange("k p -> p k"))
                b0_bc = wpre.tile([P, G4], F32)
                nc.sync.dma_start(b0_bc, _bcast_rows(b0_row[:, :], P))
                cx_sb = wpre.tile([P, KE, G4], F32)
                bc_sb = wpre.tile([P, G4], F32)

                # bc = attd_b @ W_ih.T + b_ih + b_hh, broadcast to all partitions
                # via an lhsT whose every column is the attd_b k-tile (free step 0)
                for n in range(NCH):
                    ns = slice(n * 512, (n + 1) * 512)
                    psb = ps_o.tile([P, 512], F32, tag="o512")
                    for k in range(KE):
                        nc.tensor.matmul(psb, attdb_sb[:, k:k + 1].to_broadcast([P, P]),
                                         wih_sb[:, k, ns], start=(k == 0), stop=(k == KE - 1))
                    nc.vector.tensor_add(bc_sb[:, ns], psb, b0_bc[:, ns])

                # Cx (kept in SBUF) and Ca (spilled to DRAM for phase B)
                for m in range(4):
                    psg = ps_g.tile([P, G4], F32, tag="g4")
                    g4_matmul(psg, [adwx_sb[:, k, m * P:(m + 1) * P] for k in range(KE)],
                              [wih_sb[:, k, :] for k in range(KE)])
                    nc.vector.tensor_copy(cx_sb[:, m, :], psg)
                for m in range(4):
                    psg = ps_g.tile([P, G4], F32, tag="g4")
                    g4_matmul(psg, [adwa_sb[:, k, m * P:(m + 1) * P] for k in range(KE)],
                              [wih_sb[:, k, :] for k in range(KE)])
                    sb = pre.tile([P, G4], F32, tag="big8")
                    nc.vector.tensor_copy(sb, psg)
                    nc.sync.dma_start(ca_dram[m * P:(m + 1) * P, :], sb)

                # gather + transpose x_t for this core's TPC steps
                xT_sb = xtp.tile([P, KE, rows_sh], F32)
                for j in range(TPC):
                    xg = pre.tile([P, E], F32, tag="sm2")
                    tok = pre.tile([P, 1], I32, tag="tok")
                    nc.sync.dma_start(tok, caps[j:j + 1, :].rearrange("o b -> b o"))
                    nc.gpsimd.indirect_dma_start(
                        out=xg, out_offset=None, in_=emb_W[:, :],
                        in_offset=bass.IndirectOffsetOnAxis(ap=tok[:, :1], axis=0))
                    for k in range(KE):
                        pst = ps_t.tile([P, P], F32, tag="pst")
                        nc.tensor.transpose(pst, xg[:, k * P:(k + 1) * P], identity)
                        nc.vector.tensor_copy(xT_sb[:, k, j * P:(j + 1) * P], pst)

                # PA_T shard [A, rows]: lhsT = att_Wx.T tiles, rhs = xT
                for m in range(MA):
                    for half in range(2):             # N = 1024 -> 2 x 512
                        ps = ps_o.tile([P, 512], F32, tag="o512")
                        ns = slice(half * 512, (half + 1) * 512)
                        for k in range(KE):
                            nc.tensor.matmul(ps, awx_sb[:, k, m * P:(m + 1) * P],
                                             xT_sb[:, k, ns], start=(k == 0), stop=(k == KE - 1))
                        sb = pre.tile([P, 512], F32, tag="sm2")
                        nc.vector.tensor_scalar_add(sb, ps, attb_sb[:, m:m + 1])
                        nc.sync.dma_start(pa_sh[m * P:(m + 1) * P, ns], sb)

                # PX shard [rows, 4H]: lhsT = xT tiles, rhs = Cx; + bc
                for j in range(TPC):
                    psg = ps_g.tile([P, G4], F32, tag="g4")
                    g4_matmul(psg, [xT_sb[:, k, j * P:(j + 1) * P] for k in range(KE)],
                              [cx_sb[:, k, :] for k in range(KE)])
                    sb = pre.tile([P, G4], F32, tag="big8")
                    nc.vector.tensor_tensor(sb, psg, bc_sb, op=ADD)
                    nc.sync.dma_start(px_sh[j * P:(j + 1) * P, :], sb)

                # one-time exchange
                rg = [list(range(NCORES))]
                nc.gpsimd.collective_compute("AllGather", mybir.AluOpType.bypass,
                                             replica_groups=rg, ins=[px_sh.opt()],
                                             outs=[px_all.opt()])
                nc.gpsimd.collective_compute("AllGather", mybir.AluOpType.bypass,
                                             replica_groups=rg, ins=[pa_sh.opt()],
                                             outs=[pa_all.opt()])

                # step 0: plain LSTM on features, zero initial state
                f_sb = pre.tile([P, KE, B], F32, tag="sm2")
                load_tiled(f_sb, feat_T[:, :], KE, B)
                psg = ps_g.tile([P, G4], F32, tag="g4")
                g4_matmul(psg, [f_sb[:, k, :] for k in range(KE)],
                          [wih_sb[:, k, :] for k in range(KE)])
                g0 = pre.tile([P, G4], F32, tag="big8")
                nc.vector.tensor_tensor(g0, psg, b0_bc, op=ADD)
                lstm_pointwise(g0, B, first=True, pool=pre)

            # ============ PHASE B: recurrence + output projection ============
            with tc.tile_pool(name="wloop", bufs=1) as wloop, \
                 tc.tile_pool(name="work", bufs=2) as work, \
                 tc.tile_pool(name="xstream", bufs=2) as xstream, \
                 tc.tile_pool(name="ostream", bufs=2) as ostream:
                awh_sb = wloop.tile([P, KH, A], F32)      # att_Wh.T (lhsT, F-major att)
                load_tiled(awh_sb, attWh_T[:, :], KH, A)
                whh_sb = wloop.tile([P, KH, G4], F32)     # W_hh.T (rhs for gates)
                load_tiled(whh_sb, W_hh_T[:, :], KH, G4)
                ca_sb = wloop.tile([P, KA, G4], F32)      # Ca (rhs for gates)
                load_tiled(ca_sb, ca_dram[:], KA, G4)
                owt_sb = wloop.tile([P, KH, VS], F32)     # out_W_shard.T (rhs, out-proj)
                load_tiled(owt_sb, out_WsT[:, :], KH, VS)

                def out_proj(t, nt):
                    lg = ostream.tile([P, VS], F32, tag="lg")
                    for n0 in range(0, VS, 512):
                        n1 = min(n0 + 512, VS)
                        ps = ps_o.tile([P, 512], F32, tag="o512")
                        for k in range(KH):
                            nc.tensor.matmul(ps[:, :n1 - n0], hT[:, k, :],
                                             owt_sb[:, k, n0:n1],
                                             start=(k == 0), stop=(k == KH - 1))
                        nc.vector.tensor_add(lg[:, n0:n1], ps[:, :n1 - n0],
                                             outb_bc[:, n0:n1])
                    nc.sync.dma_start(out[t, 0:nt, :], lg[0:nt, :])
                    if nt < B:
                        nc.sync.dma_start(out[t, nt:B, :], zero_out[0:B - nt, :])

                out_proj(0, int(n_t[0]))

                for t in range(1, T):
                    nt = int(n_t[t])
                    r0 = (t - 1) * B
                    cidx, j = (t - 1) // TPC, (t - 1) % TPC

                    pa_t = xstream.tile([P, KA, B], F32, tag="pa_t")
                    for k in range(KA):
                        nc.sync.dma_start(pa_t[:, k, :],
                                          pa_all[cidx, k * P:(k + 1) * P, j * B:(j + 1) * B])
                    px_t = xstream.tile([P, G4], F32, tag="px_t")
                    for n in range(NCH):
                        nc.sync.dma_start(px_t[:, n * 512:(n + 1) * 512],
                                          px_all[r0:r0 + B, n * 512:(n + 1) * 512])

                    # attention scores, feature-major: score_T [A, B]
                    sc = work.tile([P, KA, B], F32, tag="sc")
                    for m in range(MA):
                        pss = ps_s.tile([P, B], F32, tag="sc_ps")
                        for k in range(KH):
                            nc.tensor.matmul(pss, awh_sb[:, k, m * P:(m + 1) * P],
                                             hT[:, k, :], start=(k == 0), stop=(k == KH - 1))
                        nc.vector.tensor_add(sc[:, m, :], pss, pa_t[:, m, :])
                        nc.scalar.activation(sc[:, m, :], sc[:, m, :],
                                             mybir.ActivationFunctionType.Exp)
                    # softmax denominator via ones-matmul over partitions
                    psd = ps_d.tile([P, B], F32, tag="den")
                    for m in range(MA):
                        nc.tensor.matmul(psd[0:1, :], ones_col, sc[:, m, :],
                                         start=(m == 0), stop=(m == MA - 1))
                    rden = work.tile([1, B], F32, tag="rden")
                    nc.vector.reciprocal(rden, psd[0:1, :])
                    # broadcast 1/denom across partitions: K=1 matmul with all-ones lhsT row
                    dbc = ps_d.tile([P, B], F32, tag="den")
                    nc.tensor.matmul(dbc, ones_col[0:1, 0:1].to_broadcast([1, P]), rden,
                                     start=True, stop=True)
                    attn = work.tile([P, KA, B], F32, tag="attn")
                    nc.vector.tensor_mul(attn, sc, cnn_sb)
                    nc.vector.tensor_tensor(
                        attn, attn,
                        dbc.rearrange("p (k b) -> p k b", k=1).to_broadcast([P, KA, B]),
                        op=MULT)

                    # gates: G = attended @ Ca + h @ W_hh.T + PX[t]
                    psg = ps_g.tile([P, G4], F32, tag="g4")
                    g4_matmul(psg,
                              [attn[:, k, :] for k in range(KA)]
                              + [hT[:, k, :] for k in range(KH)],
                              [ca_sb[:, k, :] for k in range(KA)]
                              + [whh_sb[:, k, :] for k in range(KH)])
                    gsb = work.tile([P, G4], F32, tag="gsb")
                    nc.vector.tensor_add(gsb[0:nt, :], psg[0:nt, :], px_t[0:nt, :])

                    lstm_pointwise(gsb, nt, first=False, pool=work)
                    out_proj(t, nt)

    nc.finalize()
    return nc


def _prep_inputs(inputs):
    f = {k: np.asarray(v) for k, v in inputs.items()}
    lengths = f["lengths"].astype(np.int64)
    n_t = [int((lengths > t).sum()) for t in range(T)]

    att_W = np.asarray(f["att_W"], np.float32)
    attd_W = np.asarray(f["attd_W"], np.float32)
    W_ih = np.asarray(f["W_ih"], np.float32)
    W_hh = np.asarray(f["W_hh"], np.float32)
    out_W = np.asarray(f["out_W"], np.float32)

    base = {
        "feat_T": np.ascontiguousarray(np.asarray(f["features"], np.float32).T),
        "cnn_T": np.ascontiguousarray(np.asarray(f["cnn_features"], np.float32).T),
        "emb_W": np.ascontiguousarray(np.asarray(f["emb_W"], np.float32)),
        "W_ih_T": np.ascontiguousarray(W_ih.T),
        "W_hh_T": np.ascontiguousarray(W_hh.T),
        "b0_row": (np.asarray(f["b_ih"], np.float32)
                   + np.asarray(f["b_hh"], np.float32)).reshape(1, G4),
        "attWh_T": np.ascontiguousarray(att_W[:, E:].T),
        "attWx_T": np.ascontiguousarray(att_W[:, :E].T),
        "att_b4": np.asarray(f["att_b"], np.float32).reshape(MA, P),
        "attd_Wx": np.ascontiguousarray(attd_W[:, :E]),
        "attd_Wa": np.ascontiguousarray(attd_W[:, E:]),
        "attd_b4": np.asarray(f["attd_b"], np.float32).reshape(KE, P),
    }

    caps = np.asarray(f["captions"], np.int64)          # (B, T-1)
    caps_pad = np.zeros((NCORES * TPC, B), np.int32)
    caps_pad[:T - 1] = caps.T.astype(np.int32)          # t-major
    out_b = np.asarray(f["out_b"], np.float32)

    in_maps = []
    for c in range(NCORES):
        m = dict(base)
        m["caps"] = np.ascontiguousarray(caps_pad[c * TPC:(c + 1) * TPC])
        m["out_WsT"] = np.ascontiguousarray(out_W[c * VS:(c + 1) * VS].T)
        m["out_bs"] = np.ascontiguousarray(out_b[c * VS:(c + 1) * VS].reshape(1, VS))
        in_maps.append(m)
    return in_maps, n_t


_CACHE = {}


def kernel(**inputs):
    in_maps, n_t = _prep_inputs(inputs)
    key = tuple(n_t)
    if key not in _CACHE:
        _CACHE[key] = _build_nc(n_t)
    nc = _CACHE[key]
    res = run_bass_kernel_spmd(nc, in_maps, list(range(NCORES)))
    outs = [np.asarray(res.results[c]["out"]) for c in range(NCORES)]
    return np.concatenate(outs, axis=-1)                # (T, B, V)


# revision 13
# speedup vs baseline: 2.0014x; 2.0014x over previous
"""Trainium2 Bass kernel for nn_DecoderRNN (attention LSTM decoder + vocab projection).

Strategy (8 NeuronCores):
  - The 63-step LSTM/attention recurrence is inherently sequential and its per-step
    matmul work does not shrink with batch sharding (B=128 <= one PE M-tile), while
    per-step collectives cost >= ~5us each — so the recurrence is REPLICATED on all
    cores (identical SPMD program).
  - The dominant output projection (T*B, H) x (H, V) is sharded over the vocab
    dimension: each core computes/writes its own V/8 = 1250 logit columns in-loop.
  - All matmul operands are bf16 (fp32 PSUM accumulation, fp32 pointwise state):
    fp32 matmuls lower to two PE passes (FP32HI/LO) and draw enough power to trip
    the board throttler with 8 cores active; bf16 is one pass + fast weight load.
  - The per-timestep x-dependent GEMM inputs (attention x-part PA and the folded
    gates x-part PX = X @ (attd_Wx.T @ W_ih.T)) are precomputed SHARDED over t
    (8 steps/core) and exchanged with two one-time AllGathers.
  - attd/W_ih are folded: G = attended @ Ca + h @ W_hh.T + PX[t], with
    Ca = attd_Wa.T @ W_ih.T computed once on device.
  - Softmax normalization is deferred: attended_norm = exp(score) * cnn * (1/sum),
    with the sum taken via a ones-matmul over the feature-major exp tile.
  - Ragged lengths (sorted desc) are baked into the instruction stream: at step t
    only the first n_t rows update h/c and only those logit rows are written; the
    rest of the output is filled by DMAs from a zero tile.
"""

import os
import sys

import numpy as np

for _p in ("/opt/trn_rl_repo", "/root/.axon_site/_ro/trn_rl_repo"):
    if os.path.isdir(_p) and _p not in sys.path:
        sys.path.insert(0, _p)

import ml_dtypes
import concourse.bass as bass
import concourse.tile as tile
from concourse import bacc, mybir
from concourse.bass_utils import run_bass_kernel_spmd
from concourse.masks import make_identity

F32 = mybir.dt.float32
BF16 = mybir.dt.bfloat16
I32 = mybir.dt.int32
ADD = mybir.AluOpType.add
MULT = mybir.AluOpType.mult
NP_BF16 = ml_dtypes.bfloat16

B, T, E, H, A, V = 128, 64, 512, 512, 512, 10000
G4 = 4 * H                      # 2048
NCORES = 8
VS = V // NCORES                # 1250 vocab columns per core
TPC = 8                         # precompute t-steps per core (63 real + 1 pad)
P = 128

KE = E // P                     # 4 k-tiles over E
KH = H // P
KA = A // P
MA = A // P                     # A m-tiles (feature-major attention)
NCH = G4 // 512                 # 4 n-chunks of 512 over the gate dim


def _build_nc(n_t):
    """Build the SPMD Bass program. n_t[t] = number of active batch rows at step t
    (lengths sorted descending -> active rows are a prefix)."""
    nc = bacc.Bacc("TRN2", target_bir_lowering=False, debug=False,
                   num_devices=NCORES)

    # ---------------- I/O (bf16 for all matmul operands) ----------------
    feat_T = nc.declare_dram_parameter("feat_T", [E, B], BF16, isOutput=False)
    cnn_T = nc.declare_dram_parameter("cnn_T", [A, B], BF16, isOutput=False)
    caps = nc.declare_dram_parameter("caps", [TPC, B], I32, isOutput=False)
    emb_W = nc.declare_dram_parameter("emb_W", [V, E], BF16, isOutput=False)
    W_ih_T = nc.declare_dram_parameter("W_ih_T", [E, G4], BF16, isOutput=False)
    W_hh_T = nc.declare_dram_parameter("W_hh_T", [H, G4], BF16, isOutput=False)
    b0_row = nc.declare_dram_parameter("b0_row", [1, G4], F32, isOutput=False)
    attWh_T = nc.declare_dram_parameter("attWh_T", [H, A], BF16, isOutput=False)
    attWx_T = nc.declare_dram_parameter("attWx_T", [E, A], BF16, isOutput=False)
    att_b4 = nc.declare_dram_parameter("att_b4", [MA, P], F32, isOutput=False)
    attd_Wx = nc.declare_dram_parameter("attd_Wx", [E, E], BF16, isOutput=False)
    attd_Wa = nc.declare_dram_parameter("attd_Wa", [E, A], BF16, isOutput=False)
    attd_b4 = nc.declare_dram_parameter("attd_b4", [KE, P], BF16, isOutput=False)
    out_WsT = nc.declare_dram_parameter("out_WsT", [H, VS], BF16, isOutput=False)
    out_bs = nc.declare_dram_parameter("out_bs", [1, VS], F32, isOutput=False)
    out = nc.declare_dram_parameter("out", [T, B, VS], F32, isOutput=True)

    rows_sh = TPC * B           # 1024 precompute rows per core

    with tile.TileContext(nc) as tc:
        with (
            tc.tile_pool(name="dram", bufs=1, space="DRAM") as dramp,
            tc.tile_pool(name="consts", bufs=1) as consts,
            tc.tile_pool(name="state", bufs=1) as state,
            tc.tile_pool(name="ps_g", bufs=1, space="PSUM") as ps_g,
            tc.tile_pool(name="ps_sm", bufs=2, space="PSUM") as ps_sm,
            tc.tile_pool(name="ps_o", bufs=2, space="PSUM") as ps_o,
        ):
            # DRAM scratch
            px_sh = dramp.tile([rows_sh, G4], F32)
            pa_sh = dramp.tile([A, rows_sh], F32)
            px_all = dramp.tile([NCORES * rows_sh, G4], F32, addr_space="Shared")
            pa_all = dramp.tile([NCORES, A, rows_sh], F32, addr_space="Shared")
            ca_dram = dramp.tile([A, G4], BF16)

            def load_tiled(dst, dram_ap, ktiles, ncols, nch=512):
                """dst [P, ktiles, ncols] <- dram [(ktiles*P), ncols], one DMA per
                (k, 512-col chunk) so consumers wait on few DMA semaphores."""
                for k in range(ktiles):
                    for n0 in range(0, ncols, nch):
                        n1 = min(n0 + nch, ncols)
                        nc.sync.dma_start(dst[:, k, n0:n1],
                                          dram_ap[k * P:(k + 1) * P, n0:n1])

            # ---------------- shared constants ----------------
            ident32 = consts.tile([P, P], F32)
            make_identity(nc, ident32)
            ident16 = consts.tile([P, P], BF16)
            make_identity(nc, ident16)
            zero_out = consts.tile([P, VS], F32)
            nc.vector.memset(zero_out, 0.0)
            ones_bf = consts.tile([P, 1], BF16)
            nc.vector.memset(ones_bf, 1.0)
            cnn_sb = consts.tile([P, KA, B], BF16)    # cnn_T feature-major
            load_tiled(cnn_sb, cnn_T[:, :], KA, B)
            attb_sb = consts.tile([P, MA], F32)
            nc.sync.dma_start(attb_sb, att_b4[:, :].rearrange("m p -> p m"))
            outb_bc = consts.tile([P, VS], F32)
            nc.sync.dma_start(outb_bc, _bcast_rows(out_bs[:, :], P))

            # recurrent state (lives across both phases)
            hT = state.tile([P, KH, B], BF16)         # h transposed (feature-major)
            c_sb = state.tile([P, H], F32)            # c, B-major

            def g4_matmul(psg, lhs_list, rhs_list):
                """psg [P, G4] += sum_k lhs[k].T @ rhs[k] with N chunked to 512."""
                nk = len(lhs_list)
                for k in range(nk):
                    for n in range(NCH):
                        ns = slice(n * 512, (n + 1) * 512)
                        nc.tensor.matmul(psg[:, ns], lhs_list[k], rhs_list[k][:, ns],
                                         start=(k == 0), stop=(k == nk - 1))

            def lstm_pointwise(gsb, nt, first, pool):
                """gsb [P, 4H] pre-activation gates (B-major), activations computed
                in-place. Updates c_sb rows and hT cols [0:nt]."""
                r = slice(0, nt)
                SIG = mybir.ActivationFunctionType.Sigmoid
                TANH = mybir.ActivationFunctionType.Tanh
                nc.scalar.activation(gsb[r, 0:H], gsb[r, 0:H], SIG)              # i
                if not first:
                    nc.scalar.activation(gsb[r, H:2 * H], gsb[r, H:2 * H], SIG)  # f
                nc.scalar.activation(gsb[r, 2 * H:3 * H], gsb[r, 2 * H:3 * H], TANH)
                nc.scalar.activation(gsb[r, 3 * H:4 * H], gsb[r, 3 * H:4 * H], SIG)
                ig = pool.tile([P, H], F32, tag="ig")
                nc.vector.tensor_mul(ig[r, :], gsb[r, 0:H], gsb[r, 2 * H:3 * H])
                if first:
                    nc.vector.tensor_copy(c_sb[r, :], ig[r, :])
                else:
                    fc = pool.tile([P, H], F32, tag="fc")
                    nc.vector.tensor_mul(fc[r, :], gsb[r, H:2 * H], c_sb[r, :])
                    nc.vector.tensor_add(c_sb[r, :], fc[r, :], ig[r, :])
                tnc = pool.tile([P, H], F32, tag="tanhc")
                nc.scalar.activation(tnc[r, :], c_sb[r, :], TANH)
                h2 = pool.tile([P, H], F32, tag="h2")
                nc.vector.tensor_mul(h2[r, :], gsb[r, 3 * H:4 * H], tnc[r, :])
                for m in range(KH):
                    pst = ps_sm.tile([P, P], F32, tag="sm")
                    nc.tensor.transpose(pst, h2[:, m * P:(m + 1) * P], ident32)
                    nc.vector.tensor_copy(hT[:, m, 0:nt], pst[:, 0:nt])

            # ============ PHASE A: folds + PA/PX precompute + exchange + step 0 ============
            with tc.tile_pool(name="wpre", bufs=1) as wpre, \
                 tc.tile_pool(name="pre", bufs=2) as pre, \
                 tc.tile_pool(name="xtp", bufs=1) as xtp:
                awx_sb = wpre.tile([P, KE, A], BF16)      # att_Wx.T (lhsT for PA)
                load_tiled(awx_sb, attWx_T[:, :], KE, A)
                wih_sb = wpre.tile([P, KE, G4], BF16)     # W_ih.T (rhs)
                load_tiled(wih_sb, W_ih_T[:, :], KE, G4)
                adwx_sb = wpre.tile([P, KE, E], BF16)     # attd_Wx (lhsT for Cx)
                load_tiled(adwx_sb, attd_Wx[:, :], KE, E)
                adwa_sb = wpre.tile([P, KE, A], BF16)     # attd_Wa (lhsT for Ca)
                load_tiled(adwa_sb, attd_Wa[:, :], KE, A)
                attdb_sb = wpre.tile([P, KE], BF16)
                nc.sync.dma_start(attdb_sb, attd_b4[:, :].rearrange("k p -> p k"))
                b0_bc = wpre.tile([P, G4], F32)
                nc.sync.dma_start(b0_bc, _bcast_rows(b0_row[:, :], P))
                cx_sb = wpre.tile([P, KE, G4], BF16)
                bc_sb = wpre.tile([P, G4], F32)

                # bc = attd_b @ W_ih.T + b_ih + b_hh, broadcast to all partitions
                # via an lhsT whose every column is the attd_b k-tile (free step 0)
                for n in range(NCH):
                    ns = slice(n * 512, (n + 1) * 512)
                    psb = ps_o.tile([P, 512], F32, tag="o512")
                    for k in range(KE):
                        nc.tensor.matmul(psb, attdb_sb[:, k:k + 1].to_broadcast([P, P]),
                                         wih_sb[:, k, ns], start=(k == 0), stop=(k == KE - 1))
                    nc.vector.tensor_add(bc_sb[:, ns], psb, b0_bc[:, ns])

                # Cx (kept in SBUF) and Ca (spilled to DRAM for phase B), both bf16
                for m in range(4):
                    psg = ps_g.tile([P, G4], F32, tag="g4")
                    g4_matmul(psg, [adwx_sb[:, k, m * P:(m + 1) * P] for k in range(KE)],
                              [wih_sb[:, k, :] for k in range(KE)])
                    nc.vector.tensor_copy(cx_sb[:, m, :], psg)
                for m in range(4):
                    psg = ps_g.tile([P, G4], F32, tag="g4")
                    g4_matmul(psg, [adwa_sb[:, k, m * P:(m + 1) * P] for k in range(KE)],
                              [wih_sb[:, k, :] for k in range(KE)])
                    sb = pre.tile([P, G4], BF16, tag="ca_row")
                    nc.vector.tensor_copy(sb, psg)
                    nc.sync.dma_start(ca_dram[m * P:(m + 1) * P, :], sb)

                # gather + transpose x_t (bf16) for this core's TPC steps
                xT_sb = xtp.tile([P, KE, rows_sh], BF16)
                for j in range(TPC):
                    xg = pre.tile([P, E], BF16, tag="xg")
                    tok = pre.tile([P, 1], I32, tag="tok")
                    nc.sync.dma_start(tok, caps[j:j + 1, :].rearrange("o b -> b o"))
                    nc.gpsimd.indirect_dma_start(
                        out=xg, out_offset=None, in_=emb_W[:, :],
                        in_offset=bass.IndirectOffsetOnAxis(ap=tok[:, :1], axis=0))
                    for k in range(KE):
                        pst = ps_sm.tile([P, P], BF16, tag="sm")
                        nc.tensor.transpose(pst, xg[:, k * P:(k + 1) * P], ident16)
                        nc.vector.tensor_copy(xT_sb[:, k, j * P:(j + 1) * P], pst)

                # PA_T shard [A, rows] fp32: lhsT = att_Wx.T tiles, rhs = xT
                for m in range(MA):
                    for half in range(2):             # N = 1024 -> 2 x 512
                        ps = ps_o.tile([P, 512], F32, tag="o512")
                        ns = slice(half * 512, (half + 1) * 512)
                        for k in range(KE):
                            nc.tensor.matmul(ps, awx_sb[:, k, m * P:(m + 1) * P],
                                             xT_sb[:, k, ns], start=(k == 0), stop=(k == KE - 1))
                        sb = pre.tile([P, 512], F32, tag="pa_sb")
                        nc.vector.tensor_scalar_add(sb, ps, attb_sb[:, m:m + 1])
                        nc.sync.dma_start(pa_sh[m * P:(m + 1) * P, ns], sb)

                # PX shard [rows, 4H] fp32: lhsT = xT tiles, rhs = Cx; + bc
                for j in range(TPC):
                    psg = ps_g.tile([P, G4], F32, tag="g4")
                    g4_matmul(psg, [xT_sb[:, k, j * P:(j + 1) * P] for k in range(KE)],
                              [cx_sb[:, k, :] for k in range(KE)])
                    sb = pre.tile([P, G4], F32, tag="px_sb")
                    nc.vector.tensor_tensor(sb, psg, bc_sb, op=ADD)
                    nc.sync.dma_start(px_sh[j * P:(j + 1) * P, :], sb)

                # one-time exchange
                rg = [list(range(NCORES))]
                nc.gpsimd.collective_compute("AllGather", mybir.AluOpType.bypass,
                                             replica_groups=rg, ins=[px_sh.opt()],
                                             outs=[px_all.opt()])
                nc.gpsimd.collective_compute("AllGather", mybir.AluOpType.bypass,
                                             replica_groups=rg, ins=[pa_sh.opt()],
                                             outs=[pa_all.opt()])

                # step 0: plain LSTM on features, zero initial state
                f_sb = pre.tile([P, KE, B], BF16, tag="fT")
                load_tiled(f_sb, feat_T[:, :], KE, B)
                psg = ps_g.tile([P, G4], F32, tag="g4")
                g4_matmul(psg, [f_sb[:, k, :] for k in range(KE)],
                          [wih_sb[:, k, :] for k in range(KE)])
                g0 = pre.tile([P, G4], F32, tag="g0")
                nc.vector.tensor_tensor(g0, psg, b0_bc, op=ADD)
                lstm_pointwise(g0, B, first=True, pool=pre)

            # ============ PHASE B: recurrence + output projection ============
            with tc.tile_pool(name="wloop", bufs=1) as wloop, \
                 tc.tile_pool(name="work", bufs=2) as work, \
                 tc.tile_pool(name="xstream", bufs=2) as xstream, \
                 tc.tile_pool(name="ostream", bufs=2) as ostream:
                awh_sb = wloop.tile([P, KH, A], BF16)     # att_Wh.T (lhsT, F-major att)
                load_tiled(awh_sb, attWh_T[:, :], KH, A)
                whh_sb = wloop.tile([P, KH, G4], BF16)    # W_hh.T (rhs for gates)
                load_tiled(whh_sb, W_hh_T[:, :], KH, G4)
                ca_sb = wloop.tile([P, KA, G4], BF16)     # Ca (rhs for gates)
                load_tiled(ca_sb, ca_dram[:], KA, G4)
                owt_sb = wloop.tile([P, KH, VS], BF16)    # out_W_shard.T (rhs, out-proj)
                load_tiled(owt_sb, out_WsT[:, :], KH, VS)

                def out_proj(t, nt):
                    lg = ostream.tile([P, VS], F32, tag="lg")
                    for n0 in range(0, VS, 512):
                        n1 = min(n0 + 512, VS)
                        ps = ps_o.tile([P, 512], F32, tag="o512")
                        for k in range(KH):
                            nc.tensor.matmul(ps[:, :n1 - n0], hT[:, k, :],
                                             owt_sb[:, k, n0:n1],
                                             start=(k == 0), stop=(k == KH - 1))
                        nc.vector.tensor_add(lg[:, n0:n1], ps[:, :n1 - n0],
                                             outb_bc[:, n0:n1])
                    nc.sync.dma_start(out[t, 0:nt, :], lg[0:nt, :])
                    if nt < B:
                        nc.sync.dma_start(out[t, nt:B, :], zero_out[0:B - nt, :])

                out_proj(0, int(n_t[0]))

                for t in range(1, T):
                    nt = int(n_t[t])
                    r0 = (t - 1) * B
                    cidx, j = (t - 1) // TPC, (t - 1) % TPC

                    pa_t = xstream.tile([P, KA, B], F32, tag="pa_t")
                    for k in range(KA):
                        nc.sync.dma_start(pa_t[:, k, :],
                                          pa_all[cidx, k * P:(k + 1) * P, j * B:(j + 1) * B])
                    px_t = xstream.tile([P, G4], F32, tag="px_t")
                    for n in range(NCH):
                        nc.sync.dma_start(px_t[:, n * 512:(n + 1) * 512],
                                          px_all[r0:r0 + B, n * 512:(n + 1) * 512])

                    # attention scores, feature-major: score_T [A, B], bf16 out
                    sc = work.tile([P, KA, B], BF16, tag="sc")
                    for m in range(MA):
                        pss = ps_sm.tile([P, B], F32, tag="sm")
                        for k in range(KH):
                            nc.tensor.matmul(pss, awh_sb[:, k, m * P:(m + 1) * P],
                                             hT[:, k, :], start=(k == 0), stop=(k == KH - 1))
                        nc.vector.tensor_add(sc[:, m, :], pss, pa_t[:, m, :])
                        nc.scalar.activation(sc[:, m, :], sc[:, m, :],
                                             mybir.ActivationFunctionType.Exp)
                    # softmax denominator (row [1, B]) via ones-matmul over partitions
                    psd = ps_sm.tile([P, B], F32, tag="sm")
                    for m in range(MA):
                        nc.tensor.matmul(psd[0:1, :], ones_bf, sc[:, m, :],
                                         start=(m == 0), stop=(m == MA - 1))
                    rden = work.tile([1, B], F32, tag="rden")
                    nc.vector.reciprocal(rden, psd[0:1, :])
                    rden_bf = work.tile([1, B], BF16, tag="rdenb")
                    nc.vector.tensor_copy(rden_bf, rden)
                    # broadcast 1/denom across partitions: K=1 matmul, all-ones lhsT row
                    dbc = ps_sm.tile([P, B], F32, tag="sm")
                    nc.tensor.matmul(dbc, ones_bf[0:1, 0:1].to_broadcast([1, P]),
                                     rden_bf, start=True, stop=True)
                    dbc_sb = work.tile([P, B], BF16, tag="dbcs")
                    nc.vector.tensor_copy(dbc_sb, dbc)
                    attn = work.tile([P, KA, B], BF16, tag="attn")
                    nc.vector.tensor_mul(attn, sc, cnn_sb)
                    nc.vector.tensor_tensor(
                        attn, attn,
                        dbc_sb[:, :].rearrange("p (k b) -> p k b", k=1).to_broadcast([P, KA, B]),
                        op=MULT)

                    # gates: G = attended @ Ca + h @ W_hh.T + PX[t]
                    psg = ps_g.tile([P, G4], F32, tag="g4")
                    g4_matmul(psg,
                              [attn[:, k, :] for k in range(KA)]
                              + [hT[:, k, :] for k in range(KH)],
                              [ca_sb[:, k, :] for k in range(KA)]
                              + [whh_sb[:, k, :] for k in range(KH)])
                    gsb = work.tile([P, G4], F32, tag="gsb")
                    nc.vector.tensor_add(gsb[0:nt, :], psg[0:nt, :], px_t[0:nt, :])

                    lstm_pointwise(gsb, nt, first=False, pool=work)
                    out_proj(t, nt)

    nc.finalize()
    return nc


def _bcast_rows(dram_ap, n):
    """DMA source AP replicating a [1, N] DRAM row across n partitions."""
    return bass.AP(tensor=dram_ap.tensor, offset=dram_ap.offset,
                   ap=[[0, n]] + [list(x) for x in dram_ap.ap[1:]])


def _prep_inputs(inputs):
    f = {k: np.asarray(v) for k, v in inputs.items()}
    lengths = f["lengths"].astype(np.int64)
    n_t = [int((lengths > t).sum()) for t in range(T)]

    att_W = np.asarray(f["att_W"], np.float32)
    attd_W = np.asarray(f["attd_W"], np.float32)
    W_ih = np.asarray(f["W_ih"], np.float32)
    W_hh = np.asarray(f["W_hh"], np.float32)
    out_W = np.asarray(f["out_W"], np.float32)

    def bf(x):
        return np.ascontiguousarray(x.astype(NP_BF16))

    base = {
        "feat_T": bf(np.asarray(f["features"], np.float32).T),
        "cnn_T": bf(np.asarray(f["cnn_features"], np.float32).T),
        "emb_W": bf(np.asarray(f["emb_W"], np.float32)),
        "W_ih_T": bf(W_ih.T),
        "W_hh_T": bf(W_hh.T),
        "b0_row": (np.asarray(f["b_ih"], np.float32)
                   + np.asarray(f["b_hh"], np.float32)).reshape(1, G4),
        "attWh_T": bf(att_W[:, E:].T),
        "attWx_T": bf(att_W[:, :E].T),
        "att_b4": np.ascontiguousarray(np.asarray(f["att_b"], np.float32).reshape(MA, P)),
        "attd_Wx": bf(attd_W[:, :E]),
        "attd_Wa": bf(attd_W[:, E:]),
        "attd_b4": bf(np.asarray(f["attd_b"], np.float32).reshape(KE, P)),
    }

    caps = np.asarray(f["captions"], np.int64)          # (B, T-1)
    caps_pad = np.zeros((NCORES * TPC, B), np.int32)
    caps_pad[:T - 1] = caps.T.astype(np.int32)          # t-major
    out_b = np.asarray(f["out_b"], np.float32)

    in_maps = []
    for c in range(NCORES):
        m = dict(base)
        m["caps"] = np.ascontiguousarray(caps_pad[c * TPC:(c + 1) * TPC])
        m["out_WsT"] = bf(out_W[c * VS:(c + 1) * VS].T)
        m["out_bs"] = np.ascontiguousarray(out_b[c * VS:(c + 1) * VS].reshape(1, VS))
        in_maps.append(m)
    return in_maps, n_t


_CACHE = {}


def kernel(**inputs):
    in_maps, n_t = _prep_inputs(inputs)
    key = tuple(n_t)
    if key not in _CACHE:
        _CACHE[key] = _build_nc(n_t)
    nc = _CACHE[key]
    res = run_bass_kernel_spmd(nc, in_maps, list(range(NCORES)))
    outs = [np.asarray(res.results[c]["out"]) for c in range(NCORES)]
    return np.concatenate(outs, axis=-1)                # (T, B, V)


# revision 14
# speedup vs baseline: 2.0248x; 1.0117x over previous
"""Trainium2 Bass kernel for nn_DecoderRNN (attention LSTM decoder + vocab projection).

Strategy (8 NeuronCores):
  - The 63-step LSTM/attention recurrence is inherently sequential and its per-step
    matmul work does not shrink with batch sharding (B=128 <= one PE M-tile), while
    per-step collectives cost >= ~5us each — so the recurrence is REPLICATED on all
    cores (identical SPMD program).
  - The dominant output projection (T*B, H) x (H, V) is sharded over the vocab
    dimension: each core computes/writes its own V/8 = 1250 logit columns in-loop.
  - All matmul operands are bf16 (fp32 PSUM accumulation, fp32 pointwise state):
    fp32 matmuls lower to two PE passes (FP32HI/LO) and draw enough power to trip
    the board throttler with 8 cores active; bf16 is one pass + fast weight load.
  - Gate columns are reordered to [i|f|o|g] on the host so the LSTM pointwise phase
    needs only two ACT calls (one sigmoid over 3H, one tanh over H) — ACT calls
    have ~1us fixed cost each.
  - The per-timestep x-dependent GEMM inputs (attention x-part PA and the folded
    gates x-part PX = X @ (attd_Wx.T @ W_ih.T)) are precomputed SHARDED over t
    (8 steps/core) and exchanged with a single one-time bf16 AllGather.
  - attd/W_ih are folded: G = attended @ Ca + h @ W_hh.T + PX[t], with
    Ca = attd_Wa.T @ W_ih.T computed once on device.
  - Softmax normalization is deferred: attended_norm = exp(score) * cnn * (1/sum),
    with the sum taken via a ones-matmul over the feature-major exp tile.
  - Ragged lengths (sorted desc) are baked into the instruction stream: at step t
    only the first n_t rows update h/c and only those logit rows are written; the
    rest of the output is filled by DMAs from a zero tile.
"""

import os
import sys

import numpy as np

for _p in ("/opt/trn_rl_repo", "/root/.axon_site/_ro/trn_rl_repo"):
    if os.path.isdir(_p) and _p not in sys.path:
        sys.path.insert(0, _p)

import ml_dtypes
import concourse.bass as bass
import concourse.tile as tile
from concourse import bacc, mybir
from concourse.bass_utils import run_bass_kernel_spmd
from concourse.masks import make_identity

F32 = mybir.dt.float32
BF16 = mybir.dt.bfloat16
I32 = mybir.dt.int32
ADD = mybir.AluOpType.add
MULT = mybir.AluOpType.mult
NP_BF16 = ml_dtypes.bfloat16

B, T, E, H, A, V = 128, 64, 512, 512, 512, 10000
G4 = 4 * H                      # 2048
NCORES = 8
VS = V // NCORES                # 1250 vocab columns per core
TPC = 8                         # precompute t-steps per core (63 real + 1 pad)
P = 128

KE = E // P                     # 4 k-tiles over E
KH = H // P
KA = A // P
MA = A // P                     # A m-tiles (feature-major attention)
NCH = G4 // 512                 # 4 n-chunks of 512 over the gate dim

ROWS_SH = TPC * B               # 1024 precompute rows per core
PX_SZ = ROWS_SH * G4            # flat sizes for the merged AllGather buffer
PA_SZ = A * ROWS_SH
SH_SZ = PX_SZ + PA_SZ

# gate order after host-side reorder: [i | f | o | g]
I0, F0, O0, GG0 = 0, H, 2 * H, 3 * H


def _build_nc(n_t):
    """Build the SPMD Bass program. n_t[t] = number of active batch rows at step t
    (lengths sorted descending -> active rows are a prefix)."""
    nc = bacc.Bacc("TRN2", target_bir_lowering=False, debug=False,
                   num_devices=NCORES)

    # ---------------- I/O (bf16 for all matmul operands) ----------------
    feat_T = nc.declare_dram_parameter("feat_T", [E, B], BF16, isOutput=False)
    cnn_T = nc.declare_dram_parameter("cnn_T", [A, B], BF16, isOutput=False)
    caps = nc.declare_dram_parameter("caps", [TPC, B], I32, isOutput=False)
    emb_W = nc.declare_dram_parameter("emb_W", [V, E], BF16, isOutput=False)
    W_ih_T = nc.declare_dram_parameter("W_ih_T", [E, G4], BF16, isOutput=False)
    W_hh_T = nc.declare_dram_parameter("W_hh_T", [H, G4], BF16, isOutput=False)
    b0_row = nc.declare_dram_parameter("b0_row", [1, G4], F32, isOutput=False)
    attWh_T = nc.declare_dram_parameter("attWh_T", [H, A], BF16, isOutput=False)
    attWx_T = nc.declare_dram_parameter("attWx_T", [E, A], BF16, isOutput=False)
    att_b4 = nc.declare_dram_parameter("att_b4", [MA, P], F32, isOutput=False)
    attd_Wx = nc.declare_dram_parameter("attd_Wx", [E, E], BF16, isOutput=False)
    attd_Wa = nc.declare_dram_parameter("attd_Wa", [E, A], BF16, isOutput=False)
    attd_b4 = nc.declare_dram_parameter("attd_b4", [KE, P], BF16, isOutput=False)
    out_WsT = nc.declare_dram_parameter("out_WsT", [H, VS], BF16, isOutput=False)
    out_bs = nc.declare_dram_parameter("out_bs", [1, VS], F32, isOutput=False)
    out = nc.declare_dram_parameter("out", [T, B, VS], F32, isOutput=True)

    with tile.TileContext(nc) as tc:
        with (
            tc.tile_pool(name="dram", bufs=1, space="DRAM") as dramp,
            tc.tile_pool(name="consts", bufs=1) as consts,
            tc.tile_pool(name="state", bufs=1) as state,
            tc.tile_pool(name="ps_g", bufs=1, space="PSUM") as ps_g,    # 4 banks
            tc.tile_pool(name="ps_sm", bufs=1, space="PSUM") as ps_sm,  # 1 bank
            tc.tile_pool(name="ps_o", bufs=3, space="PSUM") as ps_o,    # 3 banks
        ):
            # DRAM scratch: per-core shard + gathered result (single merged buffer)
            sh_my = dramp.tile([SH_SZ], BF16)
            sh_all = dramp.tile([NCORES, SH_SZ], BF16, addr_space="Shared")
            ca_dram = dramp.tile([A, G4], BF16)

            def px_my_rows(j):            # [B, G4] slice of my PX shard (t-step j)
                return sh_my[j * B * G4:(j + 1) * B * G4].rearrange(
                    "(b g) -> b g", g=G4)

            def pa_my(m, ns):             # [P, len(ns)] slice of my PA shard
                return sh_my[PX_SZ:].rearrange(
                    "(a r) -> a r", r=ROWS_SH)[m * P:(m + 1) * P, ns]

            def load_tiled(dst, dram_ap, ktiles, ncols, nch=512):
                """dst [P, ktiles, ncols] <- dram [(ktiles*P), ncols] in chunks."""
                for k in range(ktiles):
                    for n0 in range(0, ncols, nch):
                        n1 = min(n0 + nch, ncols)
                        nc.sync.dma_start(dst[:, k, n0:n1],
                                          dram_ap[k * P:(k + 1) * P, n0:n1])

            # ---------------- shared constants ----------------
            ident32 = consts.tile([P, P], F32)
            make_identity(nc, ident32)
            ident16 = consts.tile([P, P], BF16)
            make_identity(nc, ident16)
            zero_out = consts.tile([P, VS], F32)
            nc.vector.memset(zero_out, 0.0)
            ones_bf = consts.tile([P, 1], BF16)
            nc.vector.memset(ones_bf, 1.0)
            cnn_sb = consts.tile([P, KA, B], BF16)    # cnn_T feature-major
            load_tiled(cnn_sb, cnn_T[:, :], KA, B)
            attb_sb = consts.tile([P, MA], F32)
            nc.sync.dma_start(attb_sb, att_b4[:, :].rearrange("m p -> p m"))
            outb_bc = consts.tile([P, VS], F32)
            nc.sync.dma_start(outb_bc, _bcast_rows(out_bs[:, :], P))

            # recurrent state (lives across both phases)
            hT = state.tile([P, KH, B], BF16)         # h transposed (feature-major)
            c_sb = state.tile([P, H], F32)            # c, B-major

            def g4_matmul(psg, lhs_list, rhs_list):
                """psg [P, G4] += sum_k lhs[k].T @ rhs[k] with N chunked to 512."""
                nk = len(lhs_list)
                for k in range(nk):
                    for n in range(NCH):
                        ns = slice(n * 512, (n + 1) * 512)
                        nc.tensor.matmul(psg[:, ns], lhs_list[k], rhs_list[k][:, ns],
                                         start=(k == 0), stop=(k == nk - 1))

            def lstm_pointwise(gsb, nt, first, pool):
                """gsb [P, 4H] pre-activation gates (B-major, [i|f|o|g] order),
                activations in-place. Updates c_sb rows and hT cols [0:nt]."""
                r = slice(0, nt)
                SIG = mybir.ActivationFunctionType.Sigmoid
                TANH = mybir.ActivationFunctionType.Tanh
                if first:   # f-gate output unused (c0 = 0); still one call
                    nc.scalar.activation(gsb[r, I0:O0 + H], gsb[r, I0:O0 + H], SIG)
                else:
                    nc.scalar.activation(gsb[r, I0:O0 + H], gsb[r, I0:O0 + H], SIG)
                nc.scalar.activation(gsb[r, GG0:GG0 + H], gsb[r, GG0:GG0 + H], TANH)
                ig = pool.tile([P, H], F32, tag="ig")
                nc.vector.tensor_mul(ig[r, :], gsb[r, I0:I0 + H], gsb[r, GG0:GG0 + H])
                if first:
                    nc.vector.tensor_copy(c_sb[r, :], ig[r, :])
                else:
                    fc = pool.tile([P, H], F32, tag="fc")
                    nc.vector.tensor_mul(fc[r, :], gsb[r, F0:F0 + H], c_sb[r, :])
                    nc.vector.tensor_add(c_sb[r, :], fc[r, :], ig[r, :])
                tnc = pool.tile([P, H], F32, tag="tanhc")
                nc.scalar.activation(tnc[r, :], c_sb[r, :], TANH)
                h2 = pool.tile([P, H], F32, tag="h2")
                nc.vector.tensor_mul(h2[r, :], gsb[r, O0:O0 + H], tnc[r, :])
                # all 4 transposes into one PSUM bank, then a single strided copy
                pst = ps_o.tile([P, 4 * P], F32, tag="o512")
                for m in range(KH):
                    nc.tensor.transpose(pst[:, m * P:(m + 1) * P],
                                        h2[:, m * P:(m + 1) * P], ident32)
                nc.vector.tensor_copy(
                    hT[:, :, 0:nt],
                    pst.rearrange("p (m b) -> p m b", m=KH)[:, :, 0:nt])

            # ============ PHASE A: folds + PA/PX precompute + exchange + step 0 ============
            with tc.tile_pool(name="wpre", bufs=1) as wpre, \
                 tc.tile_pool(name="pre", bufs=2) as pre, \
                 tc.tile_pool(name="xtp", bufs=1) as xtp:
                awx_sb = wpre.tile([P, KE, A], BF16)      # att_Wx.T (lhsT for PA)
                load_tiled(awx_sb, attWx_T[:, :], KE, A)
                wih_sb = wpre.tile([P, KE, G4], BF16)     # W_ih.T (rhs)
                load_tiled(wih_sb, W_ih_T[:, :], KE, G4)
                adwx_sb = wpre.tile([P, KE, E], BF16)     # attd_Wx (lhsT for Cx)
                load_tiled(adwx_sb, attd_Wx[:, :], KE, E)
                adwa_sb = wpre.tile([P, KE, A], BF16)     # attd_Wa (lhsT for Ca)
                load_tiled(adwa_sb, attd_Wa[:, :], KE, A)
                attdb_sb = wpre.tile([P, KE], BF16)
                nc.sync.dma_start(attdb_sb, attd_b4[:, :].rearrange("k p -> p k"))
                b0_bc = wpre.tile([P, G4], F32)
                nc.sync.dma_start(b0_bc, _bcast_rows(b0_row[:, :], P))
                cx_sb = wpre.tile([P, KE, G4], BF16)
                bc_sb = wpre.tile([P, G4], F32)

                # bc = attd_b @ W_ih.T + b_ih + b_hh, broadcast to all partitions
                # via an lhsT whose every column is the attd_b k-tile (free step 0)
                for n in range(NCH):
                    ns = slice(n * 512, (n + 1) * 512)
                    psb = ps_o.tile([P, 512], F32, tag="o512")
                    for k in range(KE):
                        nc.tensor.matmul(psb, attdb_sb[:, k:k + 1].to_broadcast([P, P]),
                                         wih_sb[:, k, ns], start=(k == 0), stop=(k == KE - 1))
                    nc.vector.tensor_add(bc_sb[:, ns], psb, b0_bc[:, ns])

                # Cx (kept in SBUF) and Ca (spilled to DRAM for phase B), both bf16
                for m in range(4):
                    psg = ps_g.tile([P, G4], F32, tag="g4")
                    g4_matmul(psg, [adwx_sb[:, k, m * P:(m + 1) * P] for k in range(KE)],
                              [wih_sb[:, k, :] for k in range(KE)])
                    nc.vector.tensor_copy(cx_sb[:, m, :], psg)
                for m in range(4):
                    psg = ps_g.tile([P, G4], F32, tag="g4")
                    g4_matmul(psg, [adwa_sb[:, k, m * P:(m + 1) * P] for k in range(KE)],
                              [wih_sb[:, k, :] for k in range(KE)])
                    sb = pre.tile([P, G4], BF16, tag="big")
                    nc.vector.tensor_copy(sb, psg)
                    nc.sync.dma_start(ca_dram[m * P:(m + 1) * P, :], sb)

                # gather + transpose x_t (bf16) for this core's TPC steps
                xT_sb = xtp.tile([P, KE, ROWS_SH], BF16)
                for j in range(TPC):
                    xg = pre.tile([P, E], BF16, tag="xg")
                    tok = pre.tile([P, 1], I32, tag="tok")
                    nc.sync.dma_start(tok, caps[j:j + 1, :].rearrange("o b -> b o"))
                    nc.gpsimd.indirect_dma_start(
                        out=xg, out_offset=None, in_=emb_W[:, :],
                        in_offset=bass.IndirectOffsetOnAxis(ap=tok[:, :1], axis=0))
                    pst = ps_o.tile([P, 4 * P], BF16, tag="o512")
                    for k in range(KE):
                        nc.tensor.transpose(pst[:, k * P:(k + 1) * P],
                                            xg[:, k * P:(k + 1) * P], ident16)
                    nc.vector.tensor_copy(
                        xT_sb[:, :, j * P:(j + 1) * P],
                        pst.rearrange("p (k b) -> p k b", k=KE))

                # PA_T shard [A, rows] bf16: lhsT = att_Wx.T tiles, rhs = xT
                for m in range(MA):
                    for half in range(2):             # N = 1024 -> 2 x 512
                        ps = ps_o.tile([P, 512], F32, tag="o512")
                        ns = slice(half * 512, (half + 1) * 512)
                        for k in range(KE):
                            nc.tensor.matmul(ps, awx_sb[:, k, m * P:(m + 1) * P],
                                             xT_sb[:, k, ns], start=(k == 0), stop=(k == KE - 1))
                        sb = pre.tile([P, 512], BF16, tag="pa_sb")
                        nc.vector.tensor_scalar_add(sb, ps, attb_sb[:, m:m + 1])
                        nc.sync.dma_start(pa_my(m, ns), sb)

                # PX shard [rows, 4H] bf16: lhsT = xT tiles, rhs = Cx; + bc
                for j in range(TPC):
                    psg = ps_g.tile([P, G4], F32, tag="g4")
                    g4_matmul(psg, [xT_sb[:, k, j * P:(j + 1) * P] for k in range(KE)],
                              [cx_sb[:, k, :] for k in range(KE)])
                    sb = pre.tile([P, G4], BF16, tag="big")
                    nc.vector.tensor_tensor(sb, psg, bc_sb, op=ADD)
                    nc.sync.dma_start(px_my_rows(j), sb)

                # one-time exchange (single merged AllGather)
                nc.gpsimd.collective_compute(
                    "AllGather", mybir.AluOpType.bypass,
                    replica_groups=[list(range(NCORES))],
                    ins=[sh_my.opt()], outs=[sh_all.opt()])

                # step 0: plain LSTM on features, zero initial state
                f_sb = pre.tile([P, KE, B], BF16, tag="fT")
                load_tiled(f_sb, feat_T[:, :], KE, B)
                psg = ps_g.tile([P, G4], F32, tag="g4")
                g4_matmul(psg, [f_sb[:, k, :] for k in range(KE)],
                          [wih_sb[:, k, :] for k in range(KE)])
                g0 = pre.tile([P, G4], F32, tag="g0")
                nc.vector.tensor_tensor(g0, psg, b0_bc, op=ADD)
                lstm_pointwise(g0, B, first=True, pool=pre)

            # ============ PHASE B: recurrence + output projection ============
            with tc.tile_pool(name="wloop", bufs=1) as wloop, \
                 tc.tile_pool(name="work", bufs=2) as work, \
                 tc.tile_pool(name="xstream", bufs=2) as xstream, \
                 tc.tile_pool(name="ostream", bufs=2) as ostream:
                awh_sb = wloop.tile([P, KH, A], BF16)     # att_Wh.T (lhsT, F-major att)
                load_tiled(awh_sb, attWh_T[:, :], KH, A)
                whh_sb = wloop.tile([P, KH, G4], BF16)    # W_hh.T (rhs for gates)
                load_tiled(whh_sb, W_hh_T[:, :], KH, G4)
                ca_sb = wloop.tile([P, KA, G4], BF16)     # Ca (rhs for gates)
                load_tiled(ca_sb, ca_dram[:], KA, G4)
                owt_sb = wloop.tile([P, KH, VS], BF16)    # out_W_shard.T (rhs, out-proj)
                load_tiled(owt_sb, out_WsT[:, :], KH, VS)

                def out_proj(t, nt):
                    lg = ostream.tile([P, VS], F32, tag="lg")
                    for n0 in range(0, VS, 512):
                        n1 = min(n0 + 512, VS)
                        ps = ps_o.tile([P, 512], F32, tag="o512")
                        for k in range(KH):
                            nc.tensor.matmul(ps[:, :n1 - n0], hT[:, k, :],
                                             owt_sb[:, k, n0:n1],
                                             start=(k == 0), stop=(k == KH - 1))
                        nc.vector.tensor_add(lg[:, n0:n1], ps[:, :n1 - n0],
                                             outb_bc[:, n0:n1])
                    nc.sync.dma_start(out[t, 0:nt, :], lg[0:nt, :])
                    if nt < B:
                        nc.sync.dma_start(out[t, nt:B, :], zero_out[0:B - nt, :])

                out_proj(0, int(n_t[0]))

                for t in range(1, T):
                    nt = int(n_t[t])
                    cidx, j = (t - 1) // TPC, (t - 1) % TPC

                    # stream this step's PA/PX slices from the gathered buffer
                    pa_t = xstream.tile([P, KA, B], BF16, tag="pa_t")
                    nc.sync.dma_start(
                        pa_t,
                        sh_all[cidx, PX_SZ:].rearrange(
                            "(k p r) -> p k r", p=P, r=ROWS_SH)[:, :, j * B:(j + 1) * B])
                    px_t = xstream.tile([P, G4], BF16, tag="px_t")
                    nc.sync.dma_start(
                        px_t,
                        sh_all[cidx, j * B * G4:(j + 1) * B * G4].rearrange(
                            "(b g) -> b g", g=G4))

                    # attention scores, feature-major: score_T [A, B] in one PSUM bank
                    pss = ps_o.tile([P, MA * B], F32, tag="o512")
                    for m in range(MA):
                        for k in range(KH):
                            nc.tensor.matmul(pss[:, m * B:(m + 1) * B],
                                             awh_sb[:, k, m * P:(m + 1) * P],
                                             hT[:, k, :], start=(k == 0), stop=(k == KH - 1))
                    sc = work.tile([P, KA, B], BF16, tag="sc")
                    nc.vector.tensor_tensor(
                        sc, pss.rearrange("p (m b) -> p m b", m=MA), pa_t, op=ADD)
                    nc.scalar.activation(sc, sc, mybir.ActivationFunctionType.Exp)

                    # softmax denominator (row [1, B]) via ones-matmul over partitions
                    psd = ps_sm.tile([P, B], F32, tag="sm")
                    for m in range(MA):
                        nc.tensor.matmul(psd[0:1, :], ones_bf, sc[:, m, :],
                                         start=(m == 0), stop=(m == MA - 1))
                    rden = work.tile([1, B], F32, tag="rden")
                    nc.vector.reciprocal(rden, psd[0:1, :])
                    rden_bf = work.tile([1, B], BF16, tag="rdenb")
                    nc.vector.tensor_copy(rden_bf, rden)
                    # broadcast 1/denom across partitions: K=1 matmul, all-ones lhsT row
                    dbc = ps_sm.tile([P, B], F32, tag="sm")
                    nc.tensor.matmul(dbc, ones_bf[0:1, 0:1].to_broadcast([1, P]),
                                     rden_bf, start=True, stop=True)
                    attn = work.tile([P, KA, B], BF16, tag="attn")
                    nc.vector.tensor_mul(attn, sc, cnn_sb)
                    nc.vector.tensor_tensor(
                        attn, attn,
                        dbc.rearrange("p (k b) -> p k b", k=1).to_broadcast([P, KA, B]),
                        op=MULT)

                    # gates: G = attended @ Ca + h @ W_hh.T + PX[t]
                    psg = ps_g.tile([P, G4], F32, tag="g4")
                    g4_matmul(psg,
                              [attn[:, k, :] for k in range(KA)]
                              + [hT[:, k, :] for k in range(KH)],
                              [ca_sb[:, k, :] for k in range(KA)]
                              + [whh_sb[:, k, :] for k in range(KH)])
                    gsb = work.tile([P, G4], F32, tag="gsb")
                    nc.vector.tensor_add(gsb[0:nt, :], psg[0:nt, :], px_t[0:nt, :])

                    lstm_pointwise(gsb, nt, first=False, pool=work)
                    out_proj(t, nt)

    nc.finalize()
    return nc


def _bcast_rows(dram_ap, n):
    """DMA source AP replicating a [1, N] DRAM row across n partitions."""
    return bass.AP(tensor=dram_ap.tensor, offset=dram_ap.offset,
                   ap=[[0, n]] + [list(x) for x in dram_ap.ap[1:]])


def _reorder_gates(w, axis):
    """Reorder the 4H gate dim from [i|f|g|o] (torch order) to [i|f|o|g]."""
    idx = np.concatenate([np.arange(0, H), np.arange(H, 2 * H),
                          np.arange(3 * H, 4 * H), np.arange(2 * H, 3 * H)])
    return np.take(w, idx, axis=axis)


def _prep_inputs(inputs):
    f = {k: np.asarray(v) for k, v in inputs.items()}
    lengths = f["lengths"].astype(np.int64)
    n_t = [int((lengths > t).sum()) for t in range(T)]

    att_W = np.asarray(f["att_W"], np.float32)
    attd_W = np.asarray(f["attd_W"], np.float32)
    W_ih = _reorder_gates(np.asarray(f["W_ih"], np.float32), axis=0)
    W_hh = _reorder_gates(np.asarray(f["W_hh"], np.float32), axis=0)
    b0 = _reorder_gates(np.asarray(f["b_ih"], np.float32)
                        + np.asarray(f["b_hh"], np.float32), axis=0)
    out_W = np.asarray(f["out_W"], np.float32)

    def bf(x):
        return np.ascontiguousarray(x.astype(NP_BF16))

    base = {
        "feat_T": bf(np.asarray(f["features"], np.float32).T),
        "cnn_T": bf(np.asarray(f["cnn_features"], np.float32).T),
        "emb_W": bf(np.asarray(f["emb_W"], np.float32)),
        "W_ih_T": bf(W_ih.T),
        "W_hh_T": bf(W_hh.T),
        "b0_row": np.ascontiguousarray(b0.reshape(1, G4)),
        "attWh_T": bf(att_W[:, E:].T),
        "attWx_T": bf(att_W[:, :E].T),
        "att_b4": np.ascontiguousarray(np.asarray(f["att_b"], np.float32).reshape(MA, P)),
        "attd_Wx": bf(attd_W[:, :E]),
        "attd_Wa": bf(attd_W[:, E:]),
        "attd_b4": bf(np.asarray(f["attd_b"], np.float32).reshape(KE, P)),
    }

    caps = np.asarray(f["captions"], np.int64)          # (B, T-1)
    caps_pad = np.zeros((NCORES * TPC, B), np.int32)
    caps_pad[:T - 1] = caps.T.astype(np.int32)          # t-major
    out_b = np.asarray(f["out_b"], np.float32)

    in_maps = []
    for c in range(NCORES):
        m = dict(base)
        m["caps"] = np.ascontiguousarray(caps_pad[c * TPC:(c + 1) * TPC])
        m["out_WsT"] = bf(out_W[c * VS:(c + 1) * VS].T)
        m["out_bs"] = np.ascontiguousarray(out_b[c * VS:(c + 1) * VS].reshape(1, VS))
        in_maps.append(m)
    return in_maps, n_t


_CACHE = {}


def kernel(**inputs):
    in_maps, n_t = _prep_inputs(inputs)
    key = tuple(n_t)
    if key not in _CACHE:
        _CACHE[key] = _build_nc(n_t)
    nc = _CACHE[key]
    res = run_bass_kernel_spmd(nc, in_maps, list(range(NCORES)))
    outs = [np.asarray(res.results[c]["out"]) for c in range(NCORES)]
    return np.concatenate(outs, axis=-1)                # (T, B, V)


# revision 19
# speedup vs baseline: 2.2665x; 1.1194x over previous
"""Trainium2 Bass kernel for nn_DecoderRNN (attention LSTM decoder + vocab projection).

Strategy (8 NeuronCores):
  - The 63-step LSTM/attention recurrence is inherently sequential and its per-step
    matmul work does not shrink with batch sharding (B=128 <= one PE M-tile), while
    per-step collectives cost >= ~5us each — so the recurrence is REPLICATED on all
    cores (identical SPMD program).
  - The dominant output projection (T*B, H) x (H, V) is sharded over the vocab
    dimension: each core computes/writes its own V/8 = 1250 logit columns in-loop.
  - All matmul operands are bf16 (fp32 PSUM accumulation, fp32 pointwise state):
    fp32 matmuls lower to two PE passes (FP32HI/LO) and draw enough power to trip
    the board throttler with 8 cores active; bf16 is one pass + fast weight load.
  - Gate columns are reordered to [i|f|o|g] on the host so the LSTM pointwise phase
    needs only two ACT calls (one sigmoid over 3H, one tanh over H) — ACT calls
    have ~1us fixed cost each.
  - The per-timestep x-dependent GEMM inputs (attention x-part PA and the folded
    gates x-part PX = X @ (attd_Wx.T @ W_ih.T)) are precomputed SHARDED over t
    (8 steps/core) and exchanged with a single one-time bf16 AllGather.
  - attd/W_ih are folded: G = attended @ Ca + h @ W_hh.T + PX[t], with
    Ca = attd_Wa.T @ W_ih.T computed once on device.
  - Softmax normalization is deferred: attended_norm = exp(score) * cnn * (1/sum),
    with the sum taken via a ones-matmul over the feature-major exp tile.
  - Ragged lengths (sorted desc) are baked into the instruction stream: at step t
    only the first n_t rows update h/c and only those logit rows are written; the
    rest of the output is filled by DMAs from a zero tile.
"""

import os
import sys

import numpy as np

for _p in ("/opt/trn_rl_repo", "/root/.axon_site/_ro/trn_rl_repo"):
    if os.path.isdir(_p) and _p not in sys.path:
        sys.path.insert(0, _p)

import ml_dtypes
import concourse.bass as bass
import concourse.tile as tile
from concourse import bacc, mybir
from concourse.bass_utils import run_bass_kernel_spmd
from concourse.masks import make_identity

F32 = mybir.dt.float32
BF16 = mybir.dt.bfloat16
I32 = mybir.dt.int32
ADD = mybir.AluOpType.add
MULT = mybir.AluOpType.mult
NP_BF16 = ml_dtypes.bfloat16

B, T, E, H, A, V = 128, 64, 512, 512, 512, 10000
G4 = 4 * H                      # 2048
NCORES = 8
VS = V // NCORES                # 1250 vocab columns per core
TPC = 8                         # precompute t-steps per core (63 real + 1 pad)
P = 128

KE = E // P                     # 4 k-tiles over E
KH = H // P
KA = A // P
MA = A // P                     # A m-tiles (feature-major attention)
NCH = G4 // 512                 # 4 n-chunks of 512 over the gate dim

ROWS_SH = TPC * B               # 1024 precompute rows per core
CHUNK = B * G4                  # 262144 elems = 512KB bf16: one AG chunk
PX_CH = TPC                     # PX shard = 8 chunks (one per t-step)
PA_CH = (A * ROWS_SH) // CHUNK  # PA shard = 2 chunks
N_CH = PX_CH + PA_CH            # 10 chunks, AllGathered individually (mesh regime)

# gate order after host-side reorder: [i | f | o | g]
I0, F0, O0, GG0 = 0, H, 2 * H, 3 * H


def _build_nc(n_t):
    """Build the SPMD Bass program. n_t[t] = number of active batch rows at step t
    (lengths sorted descending -> active rows are a prefix)."""
    nc = bacc.Bacc("TRN2", target_bir_lowering=False, debug=False,
                   num_devices=NCORES)

    # ---------------- I/O (bf16 for all matmul operands) ----------------
    feat_T = nc.declare_dram_parameter("feat_T", [E, B], BF16, isOutput=False)
    cnn_T = nc.declare_dram_parameter("cnn_T", [A, B], BF16, isOutput=False)
    caps = nc.declare_dram_parameter("caps", [TPC, B], I32, isOutput=False)
    emb_W = nc.declare_dram_parameter("emb_W", [V, E], BF16, isOutput=False)
    W_ih_T = nc.declare_dram_parameter("W_ih_T", [E, G4], BF16, isOutput=False)
    W_hh_T = nc.declare_dram_parameter("W_hh_T", [H, G4], BF16, isOutput=False)
    b0_row = nc.declare_dram_parameter("b0_row", [1, G4], F32, isOutput=False)
    attWh_T = nc.declare_dram_parameter("attWh_T", [H, A], BF16, isOutput=False)
    attWx_T = nc.declare_dram_parameter("attWx_T", [E, A], BF16, isOutput=False)
    att_b4 = nc.declare_dram_parameter("att_b4", [MA, P], F32, isOutput=False)
    attd_Wx = nc.declare_dram_parameter("attd_Wx", [E, E], BF16, isOutput=False)
    attd_Wa = nc.declare_dram_parameter("attd_Wa", [E, A], BF16, isOutput=False)
    attd_b4 = nc.declare_dram_parameter("attd_b4", [KE, P], BF16, isOutput=False)
    out_WsT = nc.declare_dram_parameter("out_WsT", [H, VS], BF16, isOutput=False)
    out_bs = nc.declare_dram_parameter("out_bs", [1, VS], F32, isOutput=False)
    out = nc.declare_dram_parameter("out", [T, B, VS], F32, isOutput=True)

    with tile.TileContext(nc) as tc:
        with (
            tc.tile_pool(name="dram", bufs=1, space="DRAM") as dramp,
            tc.tile_pool(name="consts", bufs=1) as consts,
            tc.tile_pool(name="state", bufs=1) as state,
            tc.tile_pool(name="ps_g", bufs=1, space="PSUM") as ps_g,    # 4 banks
            tc.tile_pool(name="ps_sm", bufs=1, space="PSUM") as ps_sm,  # 1 bank
            tc.tile_pool(name="ps_o", bufs=3, space="PSUM") as ps_o,    # 3 banks
        ):
            # DRAM scratch: per-core shard + gathered result (single merged buffer)
            sh_my = dramp.tile([N_CH, CHUNK], BF16)
            sh_all = [dramp.tile([NCORES, CHUNK], BF16, addr_space="Shared",
                                 name=f"sh_all_{q}")
                      for q in range(N_CH)]
            ca_dram = dramp.tile([A, G4], BF16)

            def px_my_rows(j):            # [B, G4] slice of my PX shard (t-step j)
                return sh_my[j, :].rearrange("(b g) -> b g", g=G4)

            def pa_my(m, ns):             # [P, len(ns)] slice of my PA shard
                q, mm = divmod(m, 2)      # 256 A-rows (2 m-tiles) per chunk
                return sh_my[PX_CH + q, :].rearrange(
                    "(a r) -> a r", r=ROWS_SH)[mm * P:(mm + 1) * P, ns]

            def load_tiled(dst, dram_ap, ktiles, ncols, nch=512):
                """dst [P, ktiles, ncols] <- dram [(ktiles*P), ncols] in chunks."""
                for k in range(ktiles):
                    for n0 in range(0, ncols, nch):
                        n1 = min(n0 + nch, ncols)
                        nc.sync.dma_start(dst[:, k, n0:n1],
                                          dram_ap[k * P:(k + 1) * P, n0:n1])

            # ---------------- shared constants ----------------
            ident32 = consts.tile([P, P], F32)
            make_identity(nc, ident32)
            ident16 = consts.tile([P, P], BF16)
            make_identity(nc, ident16)
            zero_out = consts.tile([P, VS], F32)
            nc.vector.memset(zero_out, 0.0)
            ones_bf = consts.tile([P, 1], BF16)
            nc.vector.memset(ones_bf, 1.0)
            cnn_sb = consts.tile([P, KA, B], BF16)    # cnn_T feature-major
            load_tiled(cnn_sb, cnn_T[:, :], KA, B)
            attb_sb = consts.tile([P, MA], F32)
            nc.sync.dma_start(attb_sb, att_b4[:, :].rearrange("m p -> p m"))
            outb_bc = consts.tile([P, VS], F32)
            nc.sync.dma_start(outb_bc, _bcast_rows(out_bs[:, :], P))

            # recurrent state (lives across both phases)
            hT = state.tile([P, KH, B], BF16)         # h transposed (feature-major)
            c_sb = state.tile([P, H], F32)            # c, B-major

            def g4_matmul(psg, lhs_list, rhs_list):
                """psg [P, G4] += sum_k lhs[k].T @ rhs[k] with N chunked to 512."""
                nk = len(lhs_list)
                for k in range(nk):
                    for n in range(NCH):
                        ns = slice(n * 512, (n + 1) * 512)
                        nc.tensor.matmul(psg[:, ns], lhs_list[k], rhs_list[k][:, ns],
                                         start=(k == 0), stop=(k == nk - 1))

            def lstm_pointwise(gsb, nt, first, pool):
                """gsb [P, 4H] pre-activation gates (B-major, [i|f|o|g] order),
                activations in-place. Updates c_sb rows and hT cols [0:nt]."""
                r = slice(0, nt)
                SIG = mybir.ActivationFunctionType.Sigmoid
                TANH = mybir.ActivationFunctionType.Tanh
                if first:   # f-gate output unused (c0 = 0); still one call
                    nc.scalar.activation(gsb[r, I0:O0 + H], gsb[r, I0:O0 + H], SIG)
                else:
                    nc.scalar.activation(gsb[r, I0:O0 + H], gsb[r, I0:O0 + H], SIG)
                nc.scalar.activation(gsb[r, GG0:GG0 + H], gsb[r, GG0:GG0 + H], TANH)
                ig = pool.tile([P, H], F32, tag="ig")
                nc.vector.tensor_mul(ig[r, :], gsb[r, I0:I0 + H], gsb[r, GG0:GG0 + H])
                if first:
                    nc.vector.tensor_copy(c_sb[r, :], ig[r, :])
                else:
                    fc = pool.tile([P, H], F32, tag="fc")
                    nc.vector.tensor_mul(fc[r, :], gsb[r, F0:F0 + H], c_sb[r, :])
                    nc.vector.tensor_add(c_sb[r, :], fc[r, :], ig[r, :])
                tnc = pool.tile([P, H], F32, tag="tanhc")
                nc.scalar.activation(tnc[r, :], c_sb[r, :], TANH)
                h2 = pool.tile([P, H], F32, tag="h2")
                nc.vector.tensor_mul(h2[r, :], gsb[r, O0:O0 + H], tnc[r, :])
                # all 4 transposes into one PSUM bank, then a single strided copy
                pst = ps_o.tile([P, 4 * P], F32, tag="o512")
                for m in range(KH):
                    nc.tensor.transpose(pst[:, m * P:(m + 1) * P],
                                        h2[:, m * P:(m + 1) * P], ident32)
                nc.vector.tensor_copy(
                    hT[:, :, 0:nt],
                    pst.rearrange("p (m b) -> p m b", m=KH)[:, :, 0:nt])

            # ============ PHASE A: folds + PA/PX precompute + exchange + step 0 ============
            with tc.tile_pool(name="wpre", bufs=1) as wpre, \
                 tc.tile_pool(name="pre", bufs=2) as pre, \
                 tc.tile_pool(name="xtp", bufs=1) as xtp:
                awx_sb = wpre.tile([P, KE, A], BF16)      # att_Wx.T (lhsT for PA)
                load_tiled(awx_sb, attWx_T[:, :], KE, A)
                wih_sb = wpre.tile([P, KE, G4], BF16)     # W_ih.T (rhs)
                load_tiled(wih_sb, W_ih_T[:, :], KE, G4)
                adwx_sb = wpre.tile([P, KE, E], BF16)     # attd_Wx (lhsT for Cx)
                load_tiled(adwx_sb, attd_Wx[:, :], KE, E)
                adwa_sb = wpre.tile([P, KE, A], BF16)     # attd_Wa (lhsT for Ca)
                load_tiled(adwa_sb, attd_Wa[:, :], KE, A)
                attdb_sb = wpre.tile([P, KE], BF16)
                nc.sync.dma_start(attdb_sb, attd_b4[:, :].rearrange("k p -> p k"))
                b0_bc = wpre.tile([P, G4], F32)
                nc.sync.dma_start(b0_bc, _bcast_rows(b0_row[:, :], P))
                cx_sb = wpre.tile([P, KE, G4], BF16)
                bc_sb = wpre.tile([P, G4], F32)

                # bc = attd_b @ W_ih.T + b_ih + b_hh, broadcast to all partitions
                # via an lhsT whose every column is the attd_b k-tile (free step 0)
                for n in range(NCH):
                    ns = slice(n * 512, (n + 1) * 512)
                    psb = ps_o.tile([P, 512], F32, tag="o512")
                    for k in range(KE):
                        nc.tensor.matmul(psb, attdb_sb[:, k:k + 1].to_broadcast([P, P]),
                                         wih_sb[:, k, ns], start=(k == 0), stop=(k == KE - 1))
                    nc.vector.tensor_add(bc_sb[:, ns], psb, b0_bc[:, ns])

                # Cx (kept in SBUF) and Ca (spilled to DRAM for phase B), both bf16
                for m in range(4):
                    psg = ps_g.tile([P, G4], F32, tag="g4")
                    g4_matmul(psg, [adwx_sb[:, k, m * P:(m + 1) * P] for k in range(KE)],
                              [wih_sb[:, k, :] for k in range(KE)])
                    nc.vector.tensor_copy(cx_sb[:, m, :], psg)
                for m in range(4):
                    psg = ps_g.tile([P, G4], F32, tag="g4")
                    g4_matmul(psg, [adwa_sb[:, k, m * P:(m + 1) * P] for k in range(KE)],
                              [wih_sb[:, k, :] for k in range(KE)])
                    sb = pre.tile([P, G4], BF16, tag="big")
                    nc.vector.tensor_copy(sb, psg)
                    nc.sync.dma_start(ca_dram[m * P:(m + 1) * P, :], sb)

                # gather + transpose x_t (bf16) for this core's TPC steps
                xT_sb = xtp.tile([P, KE, ROWS_SH], BF16)
                for j in range(TPC):
                    xg = pre.tile([P, E], BF16, tag="xg")
                    tok = pre.tile([P, 1], I32, tag="tok")
                    nc.sync.dma_start(tok, caps[j:j + 1, :].rearrange("o b -> b o"))
                    nc.gpsimd.indirect_dma_start(
                        out=xg, out_offset=None, in_=emb_W[:, :],
                        in_offset=bass.IndirectOffsetOnAxis(ap=tok[:, :1], axis=0))
                    pst = ps_o.tile([P, 4 * P], BF16, tag="o512")
                    for k in range(KE):
                        nc.tensor.transpose(pst[:, k * P:(k + 1) * P],
                                            xg[:, k * P:(k + 1) * P], ident16)
                    nc.vector.tensor_copy(
                        xT_sb[:, :, j * P:(j + 1) * P],
                        pst.rearrange("p (k b) -> p k b", k=KE))

                # PA_T shard [A, rows] bf16: lhsT = att_Wx.T tiles, rhs = xT
                for m in range(MA):
                    for half in range(2):             # N = 1024 -> 2 x 512
                        ps = ps_o.tile([P, 512], F32, tag="o512")
                        ns = slice(half * 512, (half + 1) * 512)
                        for k in range(KE):
                            nc.tensor.matmul(ps, awx_sb[:, k, m * P:(m + 1) * P],
                                             xT_sb[:, k, ns], start=(k == 0), stop=(k == KE - 1))
                        sb = pre.tile([P, 512], BF16, tag="pa_sb")
                        nc.vector.tensor_scalar_add(sb, ps, attb_sb[:, m:m + 1])
                        nc.sync.dma_start(pa_my(m, ns), sb)

                # PX shard [rows, 4H] bf16: lhsT = xT tiles, rhs = Cx; + bc
                for j in range(TPC):
                    psg = ps_g.tile([P, G4], F32, tag="g4")
                    g4_matmul(psg, [xT_sb[:, k, j * P:(j + 1) * P] for k in range(KE)],
                              [cx_sb[:, k, :] for k in range(KE)])
                    sb = pre.tile([P, G4], BF16, tag="big")
                    nc.vector.tensor_tensor(sb, psg, bc_sb, op=ADD)
                    nc.sync.dma_start(px_my_rows(j), sb)

                # one-time exchange: N_CH small AllGathers (512KB/rank each)
                # to stay in the low-latency mesh regime instead of RDH
                for q in range(N_CH):
                    nc.gpsimd.collective_compute(
                        "AllGather", mybir.AluOpType.bypass,
                        replica_groups=[list(range(NCORES))],
                        ins=[sh_my[q, :].opt()], outs=[sh_all[q][:].opt()])

                # step 0: plain LSTM on features, zero initial state
                f_sb = pre.tile([P, KE, B], BF16, tag="fT")
                load_tiled(f_sb, feat_T[:, :], KE, B)
                psg = ps_g.tile([P, G4], F32, tag="g4")
                g4_matmul(psg, [f_sb[:, k, :] for k in range(KE)],
                          [wih_sb[:, k, :] for k in range(KE)])
                g0 = pre.tile([P, G4], F32, tag="g0")
                nc.vector.tensor_tensor(g0, psg, b0_bc, op=ADD)
                lstm_pointwise(g0, B, first=True, pool=pre)

            # ============ PHASE B: recurrence + output projection ============
            with tc.tile_pool(name="wloop", bufs=1) as wloop, \
                 tc.tile_pool(name="work", bufs=2) as work, \
                 tc.tile_pool(name="xstream", bufs=2) as xstream, \
                 tc.tile_pool(name="ostream", bufs=2) as ostream:
                awh_sb = wloop.tile([P, KH, A], BF16)     # att_Wh.T (lhsT, F-major att)
                load_tiled(awh_sb, attWh_T[:, :], KH, A)
                whh_sb = wloop.tile([P, KH, G4], BF16)    # W_hh.T (rhs for gates)
                load_tiled(whh_sb, W_hh_T[:, :], KH, G4)
                ca_sb = wloop.tile([P, KA, G4], BF16)     # Ca (rhs for gates)
                load_tiled(ca_sb, ca_dram[:], KA, G4)
                owt_sb = wloop.tile([P, KH, VS], BF16)    # out_W_shard.T (rhs, out-proj)
                load_tiled(owt_sb, out_WsT[:, :], KH, VS)

                def out_proj(t, nt):
                    lg = ostream.tile([P, VS], F32, tag="lg")
                    for n0 in range(0, VS, 512):
                        n1 = min(n0 + 512, VS)
                        ps = ps_o.tile([P, 512], F32, tag="o512")
                        for k in range(KH):
                            nc.tensor.matmul(ps[:, :n1 - n0], hT[:, k, :],
                                             owt_sb[:, k, n0:n1],
                                             start=(k == 0), stop=(k == KH - 1))
                        nc.vector.tensor_add(lg[:, n0:n1], ps[:, :n1 - n0],
                                             outb_bc[:, n0:n1])
                    nc.sync.dma_start(out[t, 0:nt, :], lg[0:nt, :])
                    if nt < B:
                        nc.sync.dma_start(out[t, nt:B, :], zero_out[0:B - nt, :])

                out_proj(0, int(n_t[0]))

                for t in range(1, T):
                    nt = int(n_t[t])
                    ntp = int(n_t[t - 1])             # rows for the deferred out-proj
                    cidx, j = (t - 1) // TPC, (t - 1) % TPC

                    # stream this step's PA/PX slices from the gathered buffer
                    pa_t = xstream.tile([P, KA, B], BF16, tag="pa_t")
                    for h in range(PA_CH):
                        nc.sync.dma_start(
                            pa_t[:, h * 2:(h + 1) * 2, :],
                            sh_all[PX_CH + h][cidx, :].rearrange(
                                "(k p r) -> p k r", p=P, r=ROWS_SH)[:, :, j * B:(j + 1) * B])
                    px_t = xstream.tile([P, G4], BF16, tag="px_t")
                    nc.sync.dma_start(
                        px_t, sh_all[j][cidx, :].rearrange("(b g) -> b g", g=G4))

                    # attention scores, feature-major: score_T [A, nt] in one PSUM bank
                    pss = ps_o.tile([P, MA * B], F32, tag="o512")
                    for m in range(MA):
                        for k in range(KH):
                            nc.tensor.matmul(pss[:, m * B:m * B + nt],
                                             awh_sb[:, k, m * P:(m + 1) * P],
                                             hT[:, k, 0:nt], start=(k == 0), stop=(k == KH - 1))

                    # deferred output projection for step t-1 (hT still holds h(t-1));
                    # fills the PE while ACT/DVE run the softmax + pointwise chains
                    out_proj(t - 1, ntp)

                    sc = work.tile([P, KA, B], BF16, tag="sc")
                    nc.vector.tensor_tensor(
                        sc[:, :, 0:nt],
                        pss.rearrange("p (m b) -> p m b", m=MA)[:, :, 0:nt],
                        pa_t[:, :, 0:nt], op=ADD)
                    nc.scalar.activation(sc[:, :, 0:nt], sc[:, :, 0:nt],
                                         mybir.ActivationFunctionType.Exp)

                    # softmax denominator (row [1, nt]) via ones-matmul over partitions
                    psd = ps_sm.tile([P, B], F32, tag="sm")
                    for m in range(MA):
                        nc.tensor.matmul(psd[0:1, 0:nt], ones_bf, sc[:, m, 0:nt],
                                         start=(m == 0), stop=(m == MA - 1))
                    rden = work.tile([1, B], F32, tag="rden")
                    nc.vector.reciprocal(rden[:, 0:nt], psd[0:1, 0:nt])
                    rden_bf = work.tile([1, B], BF16, tag="rdenb")
                    nc.vector.tensor_copy(rden_bf[:, 0:nt], rden[:, 0:nt])
                    # broadcast 1/denom across partitions: K=1 matmul, all-ones lhsT row
                    dbc = ps_sm.tile([P, B], F32, tag="sm")
                    nc.tensor.matmul(dbc[:, 0:nt], ones_bf[0:1, 0:1].to_broadcast([1, P]),
                                     rden_bf[:, 0:nt], start=True, stop=True)
                    attn = work.tile([P, KA, B], BF16, tag="attn")
                    nc.vector.tensor_mul(attn[:, :, 0:nt], sc[:, :, 0:nt],
                                         cnn_sb[:, :, 0:nt])
                    nc.vector.tensor_tensor(
                        attn[:, :, 0:nt], attn[:, :, 0:nt],
                        dbc.rearrange("p (k b) -> p k b", k=1)[:, :, 0:nt]
                        .to_broadcast([P, KA, nt]),
                        op=MULT)

                    # gates: G[0:nt] = attended @ Ca + h @ W_hh.T + PX[t]
                    psg = ps_g.tile([P, G4], F32, tag="g4")
                    for ki, (lhs, rhs) in enumerate(
                            [(attn[:, k, 0:nt], ca_sb[:, k, :]) for k in range(KA)]
                            + [(hT[:, k, 0:nt], whh_sb[:, k, :]) for k in range(KH)]):
                        for n in range(NCH):
                            ns = slice(n * 512, (n + 1) * 512)
                            nc.tensor.matmul(psg[0:nt, ns], lhs, rhs[:, ns],
                                             start=(ki == 0), stop=(ki == 7))
                    gsb = work.tile([P, G4], F32, tag="gsb")
                    nc.vector.tensor_add(gsb[0:nt, 0:GG0], psg[0:nt, 0:GG0],
                                         px_t[0:nt, 0:GG0])
                    nc.vector.tensor_add(gsb[0:nt, GG0:G4], psg[0:nt, GG0:G4],
                                         px_t[0:nt, GG0:G4])

                    lstm_pointwise(gsb, nt, first=False, pool=work)

                out_proj(T - 1, int(n_t[T - 1]))

    nc.finalize()
    return nc


def _bcast_rows(dram_ap, n):
    """DMA source AP replicating a [1, N] DRAM row across n partitions."""
    return bass.AP(tensor=dram_ap.tensor, offset=dram_ap.offset,
                   ap=[[0, n]] + [list(x) for x in dram_ap.ap[1:]])


def _reorder_gates(w, axis):
    """Reorder the 4H gate dim from [i|f|g|o] (torch order) to [i|f|o|g]."""
    idx = np.concatenate([np.arange(0, H), np.arange(H, 2 * H),
                          np.arange(3 * H, 4 * H), np.arange(2 * H, 3 * H)])
    return np.take(w, idx, axis=axis)


def _prep_inputs(inputs):
    f = {k: np.asarray(v) for k, v in inputs.items()}
    lengths = f["lengths"].astype(np.int64)
    n_t = [int((lengths > t).sum()) for t in range(T)]

    att_W = np.asarray(f["att_W"], np.float32)
    attd_W = np.asarray(f["attd_W"], np.float32)
    W_ih = _reorder_gates(np.asarray(f["W_ih"], np.float32), axis=0)
    W_hh = _reorder_gates(np.asarray(f["W_hh"], np.float32), axis=0)
    b0 = _reorder_gates(np.asarray(f["b_ih"], np.float32)
                        + np.asarray(f["b_hh"], np.float32), axis=0)
    out_W = np.asarray(f["out_W"], np.float32)

    def bf(x):
        return np.ascontiguousarray(x.astype(NP_BF16))

    base = {
        "feat_T": bf(np.asarray(f["features"], np.float32).T),
        "cnn_T": bf(np.asarray(f["cnn_features"], np.float32).T),
        "emb_W": bf(np.asarray(f["emb_W"], np.float32)),
        "W_ih_T": bf(W_ih.T),
        "W_hh_T": bf(W_hh.T),
        "b0_row": np.ascontiguousarray(b0.reshape(1, G4)),
        "attWh_T": bf(att_W[:, E:].T),
        "attWx_T": bf(att_W[:, :E].T),
        "att_b4": np.ascontiguousarray(np.asarray(f["att_b"], np.float32).reshape(MA, P)),
        "attd_Wx": bf(attd_W[:, :E]),
        "attd_Wa": bf(attd_W[:, E:]),
        "attd_b4": bf(np.asarray(f["attd_b"], np.float32).reshape(KE, P)),
    }

    caps = np.asarray(f["captions"], np.int64)          # (B, T-1)
    caps_pad = np.zeros((NCORES * TPC, B), np.int32)
    caps_pad[:T - 1] = caps.T.astype(np.int32)          # t-major
    out_b = np.asarray(f["out_b"], np.float32)

    in_maps = []
    for c in range(NCORES):
        m = dict(base)
        m["caps"] = np.ascontiguousarray(caps_pad[c * TPC:(c + 1) * TPC])
        m["out_WsT"] = bf(out_W[c * VS:(c + 1) * VS].T)
        m["out_bs"] = np.ascontiguousarray(out_b[c * VS:(c + 1) * VS].reshape(1, VS))
        in_maps.append(m)
    return in_maps, n_t


_CACHE = {}


def kernel(**inputs):
    in_maps, n_t = _prep_inputs(inputs)
    key = tuple(n_t)
    if key not in _CACHE:
        _CACHE[key] = _build_nc(n_t)
    nc = _CACHE[key]
    res = run_bass_kernel_spmd(nc, in_maps, list(range(NCORES)))
    outs = [np.asarray(res.results[c]["out"]) for c in range(NCORES)]
    return np.concatenate(outs, axis=-1)                # (T, B, V)


# revision 20
# speedup vs baseline: 2.3882x; 1.0537x over previous
"""Trainium2 Bass kernel for nn_DecoderRNN (attention LSTM decoder + vocab projection).

Strategy (8 NeuronCores):
  - The 63-step LSTM/attention recurrence is inherently sequential and its per-step
    matmul work does not shrink with batch sharding (B=128 <= one PE M-tile), while
    per-step collectives cost >= ~5us each — so the recurrence is REPLICATED on all
    cores (identical SPMD program).
  - The dominant output projection (T*B, H) x (H, V) is sharded over the vocab
    dimension: each core computes/writes its own V/8 = 1250 logit columns in-loop.
  - All matmul operands are bf16 (fp32 PSUM accumulation, fp32 pointwise state):
    fp32 matmuls lower to two PE passes (FP32HI/LO) and draw enough power to trip
    the board throttler with 8 cores active; bf16 is one pass + fast weight load.
  - Gate columns are reordered to [i|f|o|g] on the host so the LSTM pointwise phase
    needs only two ACT calls (one sigmoid over 3H, one tanh over H) — ACT calls
    have ~1us fixed cost each.
  - The per-timestep x-dependent GEMM inputs (attention x-part PA and the folded
    gates x-part PX = X @ (attd_Wx.T @ W_ih.T)) are precomputed SHARDED over t
    (8 steps/core) and exchanged with a single one-time bf16 AllGather.
  - attd/W_ih are folded: G = attended @ Ca + h @ W_hh.T + PX[t], with
    Ca = attd_Wa.T @ W_ih.T computed once on device.
  - Softmax normalization is deferred: attended_norm = exp(score) * cnn * (1/sum),
    with the sum taken via a ones-matmul over the feature-major exp tile.
  - Ragged lengths (sorted desc) are baked into the instruction stream: at step t
    only the first n_t rows update h/c and only those logit rows are written; the
    rest of the output is filled by DMAs from a zero tile.
"""

import os
import sys

import numpy as np

for _p in ("/opt/trn_rl_repo", "/root/.axon_site/_ro/trn_rl_repo"):
    if os.path.isdir(_p) and _p not in sys.path:
        sys.path.insert(0, _p)

import ml_dtypes
import concourse.bass as bass
import concourse.tile as tile
from concourse import bacc, mybir
from concourse.bass_utils import run_bass_kernel_spmd
from concourse.masks import make_identity

F32 = mybir.dt.float32
BF16 = mybir.dt.bfloat16
I32 = mybir.dt.int32
ADD = mybir.AluOpType.add
MULT = mybir.AluOpType.mult
NP_BF16 = ml_dtypes.bfloat16

B, T, E, H, A, V = 128, 64, 512, 512, 512, 10000
G4 = 4 * H                      # 2048
NCORES = 8
VS = V // NCORES                # 1250 vocab columns per core
TPC = 8                         # precompute t-steps per core (63 real + 1 pad)
P = 128

KE = E // P                     # 4 k-tiles over E
KH = H // P
KA = A // P
MA = A // P                     # A m-tiles (feature-major attention)
NCH = G4 // 512                 # 4 n-chunks of 512 over the gate dim

ROWS_SH = TPC * B               # 1024 precompute rows per core
CHUNK = B * G4                  # 262144 elems = 512KB bf16: one AG chunk
PX_CH = TPC                     # PX shard = 8 chunks (one per t-step)
PA_CH = (A * ROWS_SH) // CHUNK  # PA shard = 2 chunks
N_CH = PX_CH + PA_CH            # 10 chunks, AllGathered individually (mesh regime)

# gate order after host-side reorder: [i | f | o | g]
I0, F0, O0, GG0 = 0, H, 2 * H, 3 * H


def _build_nc(n_t):
    """Build the SPMD Bass program. n_t[t] = number of active batch rows at step t
    (lengths sorted descending -> active rows are a prefix)."""
    nc = bacc.Bacc("TRN2", target_bir_lowering=False, debug=False,
                   num_devices=NCORES)

    # ---------------- I/O (bf16 for all matmul operands) ----------------
    feat_T = nc.declare_dram_parameter("feat_T", [E, B], BF16, isOutput=False)
    cnn_T = nc.declare_dram_parameter("cnn_T", [A, B], BF16, isOutput=False)
    caps = nc.declare_dram_parameter("caps", [TPC, B], I32, isOutput=False)
    emb_W = nc.declare_dram_parameter("emb_W", [V, E], BF16, isOutput=False)
    W_ih_T = nc.declare_dram_parameter("W_ih_T", [E, G4], BF16, isOutput=False)
    W_hh_T = nc.declare_dram_parameter("W_hh_T", [H, G4], BF16, isOutput=False)
    b0_row = nc.declare_dram_parameter("b0_row", [1, G4], F32, isOutput=False)
    attWh_T = nc.declare_dram_parameter("attWh_T", [H, A], BF16, isOutput=False)
    attWx_T = nc.declare_dram_parameter("attWx_T", [E, A], BF16, isOutput=False)
    att_b4 = nc.declare_dram_parameter("att_b4", [MA, P], F32, isOutput=False)
    attd_Wx = nc.declare_dram_parameter("attd_Wx", [E, E], BF16, isOutput=False)
    attd_Wa = nc.declare_dram_parameter("attd_Wa", [E, A], BF16, isOutput=False)
    attd_b4 = nc.declare_dram_parameter("attd_b4", [KE, P], BF16, isOutput=False)
    out_WsT = nc.declare_dram_parameter("out_WsT", [H, VS], BF16, isOutput=False)
    out_bs = nc.declare_dram_parameter("out_bs", [1, VS], F32, isOutput=False)
    out = nc.declare_dram_parameter("out", [T, B, VS], F32, isOutput=True)

    with tile.TileContext(nc) as tc:
        with (
            tc.tile_pool(name="dram", bufs=1, space="DRAM") as dramp,
            tc.tile_pool(name="consts", bufs=1) as consts,
            tc.tile_pool(name="state", bufs=1) as state,
            tc.tile_pool(name="ps_g", bufs=1, space="PSUM") as ps_g,    # 4 banks
            tc.tile_pool(name="ps_sm", bufs=1, space="PSUM") as ps_sm,  # 1 bank
            tc.tile_pool(name="ps_o", bufs=3, space="PSUM") as ps_o,    # 3 banks
        ):
            # DRAM scratch: per-core shard + gathered result (single merged buffer)
            sh_my = dramp.tile([N_CH, CHUNK], BF16)
            sh_all_flat = dramp.tile([NCORES, N_CH * CHUNK], BF16,
                                     addr_space="Shared")
            sh_all = [sh_all_flat[:, q * CHUNK:(q + 1) * CHUNK] for q in range(N_CH)]
            ca_dram = dramp.tile([A, G4], BF16)

            def px_my_rows(j):            # [B, G4] slice of my PX shard (t-step j)
                return sh_my[j, :].rearrange("(b g) -> b g", g=G4)

            def pa_my(m, ns):             # [P, len(ns)] slice of my PA shard
                q, mm = divmod(m, 2)      # 256 A-rows (2 m-tiles) per chunk
                return sh_my[PX_CH + q, :].rearrange(
                    "(a r) -> a r", r=ROWS_SH)[mm * P:(mm + 1) * P, ns]

            def load_tiled(dst, dram_ap, ktiles, ncols, nch=512):
                """dst [P, ktiles, ncols] <- dram [(ktiles*P), ncols] in chunks."""
                for k in range(ktiles):
                    for n0 in range(0, ncols, nch):
                        n1 = min(n0 + nch, ncols)
                        nc.sync.dma_start(dst[:, k, n0:n1],
                                          dram_ap[k * P:(k + 1) * P, n0:n1])

            # ---------------- shared constants ----------------
            ident32 = consts.tile([P, P], F32)
            make_identity(nc, ident32)
            ident16 = consts.tile([P, P], BF16)
            make_identity(nc, ident16)
            zero_out = consts.tile([P, VS], F32)
            nc.vector.memset(zero_out, 0.0)
            ones_bf = consts.tile([P, 1], BF16)
            nc.vector.memset(ones_bf, 1.0)
            cnn_sb = consts.tile([P, KA, B], BF16)    # cnn_T feature-major
            load_tiled(cnn_sb, cnn_T[:, :], KA, B)
            attb_sb = consts.tile([P, MA], F32)
            nc.sync.dma_start(attb_sb, att_b4[:, :].rearrange("m p -> p m"))
            outb_bc = consts.tile([P, VS], F32)
            nc.sync.dma_start(outb_bc, _bcast_rows(out_bs[:, :], P))

            # recurrent state (lives across both phases)
            hT = state.tile([P, KH, B], BF16)         # h transposed (feature-major)
            c_sb = state.tile([P, H], F32)            # c, B-major

            def g4_matmul(psg, lhs_list, rhs_list):
                """psg [P, G4] += sum_k lhs[k].T @ rhs[k] with N chunked to 512."""
                nk = len(lhs_list)
                for k in range(nk):
                    for n in range(NCH):
                        ns = slice(n * 512, (n + 1) * 512)
                        nc.tensor.matmul(psg[:, ns], lhs_list[k], rhs_list[k][:, ns],
                                         start=(k == 0), stop=(k == nk - 1))

            def lstm_pointwise(gsb, nt, first, pool):
                """gsb [P, 4H] pre-activation gates (B-major, [i|f|o|g] order),
                activations in-place. Updates c_sb rows and hT cols [0:nt]."""
                r = slice(0, nt)
                SIG = mybir.ActivationFunctionType.Sigmoid
                TANH = mybir.ActivationFunctionType.Tanh
                if first:   # f-gate output unused (c0 = 0); still one call
                    nc.scalar.activation(gsb[r, I0:O0 + H], gsb[r, I0:O0 + H], SIG)
                else:
                    nc.scalar.activation(gsb[r, I0:O0 + H], gsb[r, I0:O0 + H], SIG)
                nc.scalar.activation(gsb[r, GG0:GG0 + H], gsb[r, GG0:GG0 + H], TANH)
                ig = pool.tile([P, H], F32, tag="ig")
                nc.vector.tensor_mul(ig[r, :], gsb[r, I0:I0 + H], gsb[r, GG0:GG0 + H])
                if first:
                    nc.vector.tensor_copy(c_sb[r, :], ig[r, :])
                else:
                    fc = pool.tile([P, H], F32, tag="fc")
                    nc.vector.tensor_mul(fc[r, :], gsb[r, F0:F0 + H], c_sb[r, :])
                    nc.vector.tensor_add(c_sb[r, :], fc[r, :], ig[r, :])
                tnc = pool.tile([P, H], F32, tag="tanhc")
                nc.scalar.activation(tnc[r, :], c_sb[r, :], TANH)
                h2 = pool.tile([P, H], F32, tag="h2")
                nc.vector.tensor_mul(h2[r, :], gsb[r, O0:O0 + H], tnc[r, :])
                # all 4 transposes into one PSUM bank, then a single strided copy
                pst = ps_o.tile([P, 4 * P], F32, tag="o512")
                for m in range(KH):
                    nc.tensor.transpose(pst[:, m * P:(m + 1) * P],
                                        h2[:, m * P:(m + 1) * P], ident32)
                nc.vector.tensor_copy(
                    hT[:, :, 0:nt],
                    pst.rearrange("p (m b) -> p m b", m=KH)[:, :, 0:nt])

            # ============ PHASE A: folds + PA/PX precompute + exchange + step 0 ============
            with tc.tile_pool(name="wpre", bufs=1) as wpre, \
                 tc.tile_pool(name="pre", bufs=2) as pre, \
                 tc.tile_pool(name="xtp", bufs=1) as xtp:
                awx_sb = wpre.tile([P, KE, A], BF16)      # att_Wx.T (lhsT for PA)
                load_tiled(awx_sb, attWx_T[:, :], KE, A)
                wih_sb = wpre.tile([P, KE, G4], BF16)     # W_ih.T (rhs)
                load_tiled(wih_sb, W_ih_T[:, :], KE, G4)
                adwx_sb = wpre.tile([P, KE, E], BF16)     # attd_Wx (lhsT for Cx)
                load_tiled(adwx_sb, attd_Wx[:, :], KE, E)
                adwa_sb = wpre.tile([P, KE, A], BF16)     # attd_Wa (lhsT for Ca)
                load_tiled(adwa_sb, attd_Wa[:, :], KE, A)
                attdb_sb = wpre.tile([P, KE], BF16)
                nc.sync.dma_start(attdb_sb, attd_b4[:, :].rearrange("k p -> p k"))
                b0_bc = wpre.tile([P, G4], F32)
                nc.sync.dma_start(b0_bc, _bcast_rows(b0_row[:, :], P))
                cx_sb = wpre.tile([P, KE, G4], BF16)
                bc_sb = wpre.tile([P, G4], F32)

                # bc = attd_b @ W_ih.T + b_ih + b_hh, broadcast to all partitions
                # via an lhsT whose every column is the attd_b k-tile (free step 0)
                for n in range(NCH):
                    ns = slice(n * 512, (n + 1) * 512)
                    psb = ps_o.tile([P, 512], F32, tag="o512")
                    for k in range(KE):
                        nc.tensor.matmul(psb, attdb_sb[:, k:k + 1].to_broadcast([P, P]),
                                         wih_sb[:, k, ns], start=(k == 0), stop=(k == KE - 1))
                    nc.vector.tensor_add(bc_sb[:, ns], psb, b0_bc[:, ns])

                # Cx (kept in SBUF) and Ca (spilled to DRAM for phase B), both bf16
                for m in range(4):
                    psg = ps_g.tile([P, G4], F32, tag="g4")
                    g4_matmul(psg, [adwx_sb[:, k, m * P:(m + 1) * P] for k in range(KE)],
                              [wih_sb[:, k, :] for k in range(KE)])
                    nc.vector.tensor_copy(cx_sb[:, m, :], psg)
                for m in range(4):
                    psg = ps_g.tile([P, G4], F32, tag="g4")
                    g4_matmul(psg, [adwa_sb[:, k, m * P:(m + 1) * P] for k in range(KE)],
                              [wih_sb[:, k, :] for k in range(KE)])
                    sb = pre.tile([P, G4], BF16, tag="big")
                    nc.vector.tensor_copy(sb, psg)
                    nc.sync.dma_start(ca_dram[m * P:(m + 1) * P, :], sb)

                # gather + transpose x_t (bf16) for this core's TPC steps
                xT_sb = xtp.tile([P, KE, ROWS_SH], BF16)
                for j in range(TPC):
                    xg = pre.tile([P, E], BF16, tag="xg")
                    tok = pre.tile([P, 1], I32, tag="tok")
                    nc.sync.dma_start(tok, caps[j:j + 1, :].rearrange("o b -> b o"))
                    nc.gpsimd.indirect_dma_start(
                        out=xg, out_offset=None, in_=emb_W[:, :],
                        in_offset=bass.IndirectOffsetOnAxis(ap=tok[:, :1], axis=0))
                    pst = ps_o.tile([P, 4 * P], BF16, tag="o512")
                    for k in range(KE):
                        nc.tensor.transpose(pst[:, k * P:(k + 1) * P],
                                            xg[:, k * P:(k + 1) * P], ident16)
                    nc.vector.tensor_copy(
                        xT_sb[:, :, j * P:(j + 1) * P],
                        pst.rearrange("p (k b) -> p k b", k=KE))

                # PA_T shard [A, rows] bf16: lhsT = att_Wx.T tiles, rhs = xT
                for m in range(MA):
                    for half in range(2):             # N = 1024 -> 2 x 512
                        ps = ps_o.tile([P, 512], F32, tag="o512")
                        ns = slice(half * 512, (half + 1) * 512)
                        for k in range(KE):
                            nc.tensor.matmul(ps, awx_sb[:, k, m * P:(m + 1) * P],
                                             xT_sb[:, k, ns], start=(k == 0), stop=(k == KE - 1))
                        sb = pre.tile([P, 512], BF16, tag="pa_sb")
                        nc.vector.tensor_scalar_add(sb, ps, attb_sb[:, m:m + 1])
                        nc.sync.dma_start(pa_my(m, ns), sb)

                # PX shard [rows, 4H] bf16: lhsT = xT tiles, rhs = Cx; + bc
                for j in range(TPC):
                    psg = ps_g.tile([P, G4], F32, tag="g4")
                    g4_matmul(psg, [xT_sb[:, k, j * P:(j + 1) * P] for k in range(KE)],
                              [cx_sb[:, k, :] for k in range(KE)])
                    sb = pre.tile([P, G4], BF16, tag="big")
                    nc.vector.tensor_tensor(sb, psg, bc_sb, op=ADD)
                    nc.sync.dma_start(px_my_rows(j), sb)

                # one-time exchange (single AllGather; chunked variants hit the
                # same RDH algorithm with a ~40us floor per call and lose)
                nc.gpsimd.collective_compute(
                    "AllGather", mybir.AluOpType.bypass,
                    replica_groups=[list(range(NCORES))],
                    ins=[sh_my[:, :].opt()], outs=[sh_all_flat[:, :].opt()])

                # step 0: plain LSTM on features, zero initial state
                f_sb = pre.tile([P, KE, B], BF16, tag="fT")
                load_tiled(f_sb, feat_T[:, :], KE, B)
                psg = ps_g.tile([P, G4], F32, tag="g4")
                g4_matmul(psg, [f_sb[:, k, :] for k in range(KE)],
                          [wih_sb[:, k, :] for k in range(KE)])
                g0 = pre.tile([P, G4], F32, tag="g0")
                nc.vector.tensor_tensor(g0, psg, b0_bc, op=ADD)
                lstm_pointwise(g0, B, first=True, pool=pre)

            # ============ PHASE B: recurrence + output projection ============
            with tc.tile_pool(name="wloop", bufs=1) as wloop, \
                 tc.tile_pool(name="work", bufs=2) as work, \
                 tc.tile_pool(name="xstream", bufs=2) as xstream, \
                 tc.tile_pool(name="ostream", bufs=2) as ostream:
                awh_sb = wloop.tile([P, KH, A], BF16)     # att_Wh.T (lhsT, F-major att)
                load_tiled(awh_sb, attWh_T[:, :], KH, A)
                whh_sb = wloop.tile([P, KH, G4], BF16)    # W_hh.T (rhs for gates)
                load_tiled(whh_sb, W_hh_T[:, :], KH, G4)
                ca_sb = wloop.tile([P, KA, G4], BF16)     # Ca (rhs for gates)
                load_tiled(ca_sb, ca_dram[:], KA, G4)
                owt_sb = wloop.tile([P, KH, VS], BF16)    # out_W_shard.T (rhs, out-proj)
                load_tiled(owt_sb, out_WsT[:, :], KH, VS)

                def out_proj(t, nt):
                    lg = ostream.tile([P, VS], F32, tag="lg")
                    for n0 in range(0, VS, 512):
                        n1 = min(n0 + 512, VS)
                        ps = ps_o.tile([P, 512], F32, tag="o512")
                        for k in range(KH):
                            nc.tensor.matmul(ps[:, :n1 - n0], hT[:, k, :],
                                             owt_sb[:, k, n0:n1],
                                             start=(k == 0), stop=(k == KH - 1))
                        nc.vector.tensor_add(lg[:, n0:n1], ps[:, :n1 - n0],
                                             outb_bc[:, n0:n1])
                    nc.sync.dma_start(out[t, 0:nt, :], lg[0:nt, :])
                    if nt < B:
                        nc.sync.dma_start(out[t, nt:B, :], zero_out[0:B - nt, :])

                out_proj(0, int(n_t[0]))

                for t in range(1, T):
                    nt = int(n_t[t])
                    ntp = int(n_t[t - 1])             # rows for the deferred out-proj
                    cidx, j = (t - 1) // TPC, (t - 1) % TPC

                    # stream this step's PA/PX slices from the gathered buffer
                    pa_t = xstream.tile([P, KA, B], BF16, tag="pa_t")
                    for h in range(PA_CH):
                        nc.sync.dma_start(
                            pa_t[:, h * 2:(h + 1) * 2, :],
                            sh_all[PX_CH + h][cidx, :].rearrange(
                                "(k p r) -> p k r", p=P, r=ROWS_SH)[:, :, j * B:(j + 1) * B])
                    px_t = xstream.tile([P, G4], BF16, tag="px_t")
                    nc.sync.dma_start(
                        px_t, sh_all[j][cidx, :].rearrange("(b g) -> b g", g=G4))

                    # attention scores, feature-major: score_T [A, nt] in one PSUM bank
                    pss = ps_o.tile([P, MA * B], F32, tag="o512")
                    for m in range(MA):
                        for k in range(KH):
                            nc.tensor.matmul(pss[:, m * B:m * B + nt],
                                             awh_sb[:, k, m * P:(m + 1) * P],
                                             hT[:, k, 0:nt], start=(k == 0), stop=(k == KH - 1))

                    # deferred output projection for step t-1 (hT still holds h(t-1));
                    # fills the PE while ACT/DVE run the softmax + pointwise chains
                    out_proj(t - 1, ntp)

                    sc = work.tile([P, KA, B], BF16, tag="sc")
                    nc.vector.tensor_tensor(
                        sc[:, :, 0:nt],
                        pss.rearrange("p (m b) -> p m b", m=MA)[:, :, 0:nt],
                        pa_t[:, :, 0:nt], op=ADD)
                    nc.scalar.activation(sc[:, :, 0:nt], sc[:, :, 0:nt],
                                         mybir.ActivationFunctionType.Exp)

                    # softmax denominator (row [1, nt]) via ones-matmul over partitions
                    psd = ps_sm.tile([P, B], F32, tag="sm")
                    for m in range(MA):
                        nc.tensor.matmul(psd[0:1, 0:nt], ones_bf, sc[:, m, 0:nt],
                                         start=(m == 0), stop=(m == MA - 1))
                    rden = work.tile([1, B], F32, tag="rden")
                    nc.vector.reciprocal(rden[:, 0:nt], psd[0:1, 0:nt])
                    rden_bf = work.tile([1, B], BF16, tag="rdenb")
                    nc.vector.tensor_copy(rden_bf[:, 0:nt], rden[:, 0:nt])
                    # broadcast 1/denom across partitions: K=1 matmul, all-ones lhsT row
                    dbc = ps_sm.tile([P, B], F32, tag="sm")
                    nc.tensor.matmul(dbc[:, 0:nt], ones_bf[0:1, 0:1].to_broadcast([1, P]),
                                     rden_bf[:, 0:nt], start=True, stop=True)
                    attn = work.tile([P, KA, B], BF16, tag="attn")
                    nc.vector.tensor_mul(attn[:, :, 0:nt], sc[:, :, 0:nt],
                                         cnn_sb[:, :, 0:nt])
                    nc.vector.tensor_tensor(
                        attn[:, :, 0:nt], attn[:, :, 0:nt],
                        dbc.rearrange("p (k b) -> p k b", k=1)[:, :, 0:nt]
                        .to_broadcast([P, KA, nt]),
                        op=MULT)

                    # gates: G[0:nt] = attended @ Ca + h @ W_hh.T + PX[t]
                    psg = ps_g.tile([P, G4], F32, tag="g4")
                    for ki, (lhs, rhs) in enumerate(
                            [(attn[:, k, 0:nt], ca_sb[:, k, :]) for k in range(KA)]
                            + [(hT[:, k, 0:nt], whh_sb[:, k, :]) for k in range(KH)]):
                        for n in range(NCH):
                            ns = slice(n * 512, (n + 1) * 512)
                            nc.tensor.matmul(psg[0:nt, ns], lhs, rhs[:, ns],
                                             start=(ki == 0), stop=(ki == 7))
                    gsb = work.tile([P, G4], F32, tag="gsb")
                    nc.vector.tensor_add(gsb[0:nt, 0:GG0], psg[0:nt, 0:GG0],
                                         px_t[0:nt, 0:GG0])
                    nc.vector.tensor_add(gsb[0:nt, GG0:G4], psg[0:nt, GG0:G4],
                                         px_t[0:nt, GG0:G4])

                    lstm_pointwise(gsb, nt, first=False, pool=work)

                out_proj(T - 1, int(n_t[T - 1]))

    nc.finalize()
    return nc


def _bcast_rows(dram_ap, n):
    """DMA source AP replicating a [1, N] DRAM row across n partitions."""
    return bass.AP(tensor=dram_ap.tensor, offset=dram_ap.offset,
                   ap=[[0, n]] + [list(x) for x in dram_ap.ap[1:]])


def _reorder_gates(w, axis):
    """Reorder the 4H gate dim from [i|f|g|o] (torch order) to [i|f|o|g]."""
    idx = np.concatenate([np.arange(0, H), np.arange(H, 2 * H),
                          np.arange(3 * H, 4 * H), np.arange(2 * H, 3 * H)])
    return np.take(w, idx, axis=axis)


def _prep_inputs(inputs):
    f = {k: np.asarray(v) for k, v in inputs.items()}
    lengths = f["lengths"].astype(np.int64)
    n_t = [int((lengths > t).sum()) for t in range(T)]

    att_W = np.asarray(f["att_W"], np.float32)
    attd_W = np.asarray(f["attd_W"], np.float32)
    W_ih = _reorder_gates(np.asarray(f["W_ih"], np.float32), axis=0)
    W_hh = _reorder_gates(np.asarray(f["W_hh"], np.float32), axis=0)
    b0 = _reorder_gates(np.asarray(f["b_ih"], np.float32)
                        + np.asarray(f["b_hh"], np.float32), axis=0)
    out_W = np.asarray(f["out_W"], np.float32)

    def bf(x):
        return np.ascontiguousarray(x.astype(NP_BF16))

    base = {
        "feat_T": bf(np.asarray(f["features"], np.float32).T),
        "cnn_T": bf(np.asarray(f["cnn_features"], np.float32).T),
        "emb_W": bf(np.asarray(f["emb_W"], np.float32)),
        "W_ih_T": bf(W_ih.T),
        "W_hh_T": bf(W_hh.T),
        "b0_row": np.ascontiguousarray(b0.reshape(1, G4)),
        "attWh_T": bf(att_W[:, E:].T),
        "attWx_T": bf(att_W[:, :E].T),
        "att_b4": np.ascontiguousarray(np.asarray(f["att_b"], np.float32).reshape(MA, P)),
        "attd_Wx": bf(attd_W[:, :E]),
        "attd_Wa": bf(attd_W[:, E:]),
        "attd_b4": bf(np.asarray(f["attd_b"], np.float32).reshape(KE, P)),
    }

    caps = np.asarray(f["captions"], np.int64)          # (B, T-1)
    caps_pad = np.zeros((NCORES * TPC, B), np.int32)
    caps_pad[:T - 1] = caps.T.astype(np.int32)          # t-major
    out_b = np.asarray(f["out_b"], np.float32)

    in_maps = []
    for c in range(NCORES):
        m = dict(base)
        m["caps"] = np.ascontiguousarray(caps_pad[c * TPC:(c + 1) * TPC])
        m["out_WsT"] = bf(out_W[c * VS:(c + 1) * VS].T)
        m["out_bs"] = np.ascontiguousarray(out_b[c * VS:(c + 1) * VS].reshape(1, VS))
        in_maps.append(m)
    return in_maps, n_t


_CACHE = {}


def kernel(**inputs):
    in_maps, n_t = _prep_inputs(inputs)
    key = tuple(n_t)
    if key not in _CACHE:
        _CACHE[key] = _build_nc(n_t)
    nc = _CACHE[key]
    res = run_bass_kernel_spmd(nc, in_maps, list(range(NCORES)))
    outs = [np.asarray(res.results[c]["out"]) for c in range(NCORES)]
    return np.concatenate(outs, axis=-1)                # (T, B, V)
